# revision 1
# baseline (speedup 1.0000x reference)
"""Trainium2 Bass kernel for nn_JointMamba: 4-direction Mamba scan + GLU conv.

Sharding: phase 1 runs the 8 independent (batch, direction) Mamba blocks one
per NeuronCore; phase 2 reshards the merged feature maps over (image, row-half)
and runs the 3x3 GLU conv, one shard per core. Host does only permutations /
layout prep (scan_jego / merge_jego are pure index shuffles).
"""
import sys
import numpy as np

try:
    import concourse.bass as bass  # noqa: F401
except ImportError:
    sys.path.insert(0, "/opt/trn_rl_repo")

import concourse.bass as bass
import concourse.bacc as bacc
import concourse.mybir as mybir
from concourse.bass_utils import run_bass_kernel_spmd
from concourse import tile

F32 = mybir.dt.float32
BF16 = mybir.dt.bfloat16
ALU = mybir.AluOpType
AF = mybir.ActivationFunctionType

B, C, H8, W8 = 2, 256, 64, 64
D_INNER, D_STATE, D_CONV, DT_RANK = 512, 16, 4, 16
L = (H8 // 2) * W8  # 2048
EPS = 1e-5

_cache = {}


def _bf16(x):
    import ml_dtypes
    return np.asarray(x, dtype=ml_dtypes.bfloat16)


# ---------------------------------------------------------------------------
# host-side permutations (pure data movement)
# ---------------------------------------------------------------------------

def scan_jego_np(d0, d1):
    d2w = np.concatenate([d0, d1], 3)
    d2h = np.concatenate([d0, d1], 2)
    b, c = d0.shape[:2]
    x0 = d2w[:, :, ::2, ::2].reshape(b, c, -1)
    x1 = np.swapaxes(d2h, 2, 3)[:, :, 1::2, 1::2].reshape(b, c, -1)
    x2 = d2w[:, :, ::2, 1::2].reshape(b, c, -1)[:, :, ::-1]
    x3 = np.swapaxes(d2h, 2, 3)[:, :, ::2, 1::2].reshape(b, c, -1)[:, :, ::-1]
    return np.stack([x0, x1, x2, x3], 1)  # [B,4,C,L]


def merge_jego_np(ys, ori_h, ori_w):
    b, k, c, Lx = ys.shape
    H, W = ori_h // 2, ori_w // 2
    y2w = np.zeros((b, c, ori_h, 2 * ori_w), ys.dtype)
    y2h = np.zeros((b, c, 2 * ori_h, ori_w), ys.dtype)
    y2w[:, :, ::2, ::2] = ys[:, 0].reshape(b, c, H, 2 * W)
    y2h[:, :, 1::2, 1::2] = np.swapaxes(ys[:, 1].reshape(b, c, W, 2 * H), 2, 3)
    y2w[:, :, ::2, 1::2] = ys[:, 2][:, :, ::-1].reshape(b, c, H, 2 * W)
    y2h[:, :, 1::2, ::2] = np.swapaxes(ys[:, 3][:, :, ::-1].reshape(b, c, W, 2 * H), 2, 3)
    d0w, d1w = np.split(y2w, 2, axis=3)
    d0h, d1h = np.split(y2h, 2, axis=2)
    return d0w + d0h, d1w + d1h


# ---------------------------------------------------------------------------
# phase 2: 3x3 conv + GLU, sharded over (image, row-half)
# ---------------------------------------------------------------------------

def build_phase2():
    """Per core: dpad [256,34,66] f32, wc [9,2,128,512] bf16 (lhsT per tap),
    wlo [9,2,128,512] bf16 (error-compensation term), bias [512] f32.
    Output o [256, 2048] f32 (= [256, 32, 64] GLU'd rows)."""
    nc = bacc.Bacc("TRN2", target_bir_lowering=False, debug=False, num_devices=8)
    dpad = nc.dram_tensor("dpad", [2, 128, 34 * 66], F32, kind="ExternalInput")
    wc = nc.dram_tensor("wc", [9, 2, 128, 512], BF16, kind="ExternalInput")
    wlo = nc.dram_tensor("wlo", [9, 2, 128, 512], BF16, kind="ExternalInput")
    bias = nc.dram_tensor("bias", [128, 4], F32, kind="ExternalInput")
    out = nc.dram_tensor("o", [2, 128, 2048], F32, kind="ExternalOutput")

    with tile.TileContext(nc) as tc:
        with tc.tile_pool(name="cw", bufs=1) as cw, \
             tc.tile_pool(name="cd", bufs=1) as cd, \
             tc.tile_pool(name="cpsum", bufs=2, space="PSUM") as cpsum, \
             tc.tile_pool(name="cact", bufs=3) as cact, \
             tc.tile_pool(name="cdbf", bufs=1) as cdbf:
            dt_ = []
            for kc in range(2):
                d = cd.tile([128, 34 * 66], F32, name=f"d{kc}")
                nc.sync.dma_start(out=d[:], in_=dpad[kc])
                dt_.append(d)
            # bf16 copy of the input for the hi-term matmuls
            dbf = []
            for kc in range(2):
                db = cdbf.tile([128, 34 * 66], BF16, name=f"db{kc}")
                nc.scalar.activation(out=db[:], in_=dt_[kc][:], func=AF.Copy)
                dbf.append(db)
            # residual input: dlo = d - bf16(d) (computed on DVE)
            dlo = []
            for kc in range(2):
                dl = cdbf.tile([128, 34 * 66], BF16, name=f"dl{kc}")
                nc.vector.scalar_tensor_tensor(
                    out=dl[:], in0=dbf[kc][:], scalar=-1.0, in1=dt_[kc][:],
                    op0=ALU.mult, op1=ALU.add)
                dlo.append(dl)
            wt = []
            wlo_t = []
            for tap in range(9):
                row_w = []
                row_l = []
                for kc in range(2):
                    w_ = cw.tile([128, 512], BF16, name=f"w{tap}_{kc}")
                    nc.sync.dma_start(out=w_[:], in_=wc[tap, kc])
                    row_w.append(w_)
                    wl_ = cw.tile([128, 512], BF16, name=f"wl{tap}_{kc}")
                    nc.sync.dma_start(out=wl_[:], in_=wlo[tap, kc])
                    row_l.append(wl_)
                wt.append(row_w)
                wlo_t.append(row_l)
            bias_t = cw.tile([128, 4], F32, name="bias_t")
            nc.sync.dma_start(out=bias_t[:], in_=bias[:])

            for rg in range(4):  # row groups of 8 output rows
                ps = []
                for m in range(4):  # co tiles of 128
                    p = cpsum.tile([128, 512], F32, name=f"ps{m}")
                    ps.append(p)
                    first = True
                    for tap in range(9):
                        dy, dx = divmod(tap, 3)
                        for kc in range(2):
                            rhs = dt_[kc][:, (rg * 8 + dy) * 66 + dx:]
                            rhs = bass.AP(rhs.tensor, rhs.offset,
                                          [rhs.ap[0], [66, 8], [1, 64]])
                            rhs_bf = dbf[kc][:, (rg * 8 + dy) * 66 + dx:]
                            rhs_bf = bass.AP(rhs_bf.tensor, rhs_bf.offset,
                                             [rhs_bf.ap[0], [66, 8], [1, 64]])
                            # hi term: bf16(w) @ bf16(d)
                            nc.tensor.matmul(
                                p[:], lhsT=wt[tap][kc][:, m * 128:(m + 1) * 128],
                                rhs=rhs_bf, start=first, stop=False)
                            first = False
                            # compensation: wlo @ bf16(d) + bf16(w) @ dlo
                            rhs_lo = dlo[kc][:, (rg * 8 + dy) * 66 + dx:]
                            rhs_lo = bass.AP(rhs_lo.tensor, rhs_lo.offset,
                                             [rhs_lo.ap[0], [66, 8], [1, 64]])
                            nc.tensor.matmul(
                                p[:], lhsT=wlo_t[tap][kc][:, m * 128:(m + 1) * 128],
                                rhs=rhs_bf, start=False, stop=False)
                            last = (tap == 8 and kc == 1)
                            nc.tensor.matmul(
                                p[:], lhsT=wt[tap][kc][:, m * 128:(m + 1) * 128],
                                rhs=rhs_lo, start=False, stop=last)
                # GLU: a = ps[0..1], g = ps[2..3]
                for m in range(2):
                    sg = cact.tile([128, 512], F32, name="sg")
                    nc.scalar.activation(out=sg[:], in_=ps[2 + m][:],
                                         func=AF.Sigmoid, bias=bias_t[:, 2 + m:3 + m])
                    av = cact.tile([128, 512], F32, name="av")
                    nc.scalar.activation(out=av[:], in_=ps[m][:],
                                         func=AF.Identity, bias=bias_t[:, m:m + 1])
                    og = cact.tile([128, 512], F32, name="og")
                    nc.vector.tensor_tensor(out=og[:], in0=av[:], in1=sg[:], op=ALU.mult)
                    nc.sync.dma_start(out=out[m, :, rg * 512:(rg + 1) * 512], in_=og[:])
    nc.compile()
    return nc


def prep_phase2_weights(glu_w, glu_b):
    # wc[tap, kc, ci, co] = glu_w[co, kc*128+ci, dy, dx]
    w = np.transpose(glu_w, (2, 3, 1, 0)).reshape(9, 2, 128, 512)
    w_hi = _bf16(w)
    w_lo = _bf16(w - np.asarray(w_hi, np.float32))
    bias = glu_b.reshape(4, 128).T.copy()  # [128, 4] per-partition
    return w_hi, w_lo, bias


def run_phase2(Dfull, glu_w, glu_b):
    """Dfull [4, 256, 64, 64] -> [4, 256, 64, 64] after conv+GLU."""
    if "p2" not in _cache:
        _cache["p2"] = build_phase2()
    nc = _cache["p2"]
    w_hi, w_lo, bias = prep_phase2_weights(glu_w, glu_b)
    Dpad = np.pad(Dfull, ((0, 0), (0, 0), (1, 1), (1, 1)))
    ins = []
    for core in range(8):
        img, half = divmod(core, 2)
        r0 = half * 32
        dslice = Dpad[img, :, r0:r0 + 34, :].reshape(2, 128, 34 * 66)
        ins.append({"dpad": np.ascontiguousarray(dslice), "wc": w_hi,
                    "wlo": w_lo, "bias": bias})
    res = run_bass_kernel_spmd(nc, ins, list(range(8)))
    out = np.zeros((4, 256, 64, 64), np.float32)
    for core in range(8):
        img, half = divmod(core, 2)
        o = res.results[core]["o"].reshape(256, 32, 64)
        out[img, :, half * 32:half * 32 + 32, :] = o
    return out


# ---------------------------------------------------------------------------
# phase 1: per-(b,k) Mamba block on one core
# layout: feature-major ([channel, t]) throughout; selective scan uses the
# native DVE TensorTensorScan along the free (t) axis, 16 state rows per
# d-block handled as independent [128, T] sweeps.
# ---------------------------------------------------------------------------

def build_phase1():
    nc = bacc.Bacc("TRN2", target_bir_lowering=False, debug=False, num_devices=8)
    xT = nc.dram_tensor("xT", [2, 128, L], F32, kind="ExternalInput")
    nwb = nc.dram_tensor("nwb", [2, 128, 2], F32, kind="ExternalInput")      # nw, nb
    inwT = nc.dram_tensor("inwT", [2, 128, 2 * D_INNER], BF16, kind="ExternalInput")
    convw = nc.dram_tensor("convw", [4, 128, D_CONV], F32, kind="ExternalInput")
    convb = nc.dram_tensor("convb", [4, 128, 1], F32, kind="ExternalInput")
    xprojT = nc.dram_tensor("xprojT", [4, 128, 48], BF16, kind="ExternalInput")
    dtwT = nc.dram_tensor("dtwT", [16, D_INNER], BF16, kind="ExternalInput")
    dtb = nc.dram_tensor("dtb", [4, 128, 1], F32, kind="ExternalInput")
    AT = nc.dram_tensor("AT", [4, 128, D_STATE], F32, kind="ExternalInput")
    Dpt = nc.dram_tensor("Dpt", [4, 128, 1], F32, kind="ExternalInput")
    outwT = nc.dram_tensor("outwT", [4, 128, C], BF16, kind="ExternalInput")
    ones1 = nc.dram_tensor("ones1", [1, 128], BF16, kind="ExternalInput")
    oneM = nc.dram_tensor("oneM", [128, 128], BF16, kind="ExternalInput")    # 1/256
    outT = nc.dram_tensor("outT", [2, 128, L], F32, kind="ExternalOutput")

    TT = 512  # t-tile for the scan stage (SBUF-bound)
    with tile.TileContext(nc) as tc:
        import contextlib
        stack = contextlib.ExitStack()
        wpool = stack.enter_context(tc.tile_pool(name="wpool", bufs=1))
        big = stack.enter_context(tc.tile_pool(name="big", bufs=1))
        ps = stack.enter_context(tc.tile_pool(name="ps", bufs=2, space="PSUM"))
        ps4 = stack.enter_context(tc.tile_pool(name="ps4", bufs=2, space="PSUM"))
        scr = stack.enter_context(tc.tile_pool(name="scr", bufs=2))
        bc = stack.enter_context(tc.tile_pool(name="bc", bufs=1))
        sc = stack.enter_context(tc.tile_pool(name="sc", bufs=3))

        # ---- load inputs
        x_t = [wpool.tile([128, L], F32, name=f"x{i}") for i in range(2)]
        for i in range(2):
            nc.sync.dma_start(out=x_t[i][:], in_=xT[i])
        nwb_t = wpool.tile([128, 4], F32, name="nwb_t")
        for i in range(2):
            nc.sync.dma_start(out=nwb_t[:, 2 * i:2 * i + 2], in_=nwb[i])
        inw_t = [wpool.tile([128, 2 * D_INNER], BF16, name=f"inw{i}") for i in range(2)]
        for i in range(2):
            nc.sync.dma_start(out=inw_t[i][:], in_=inwT[i])
        convw_t = [wpool.tile([128, D_CONV], F32, name=f"cw{i}") for i in range(4)]
        convb_t = [wpool.tile([128, 1], F32, name=f"cb{i}") for i in range(4)]
        xproj_t = [wpool.tile([128, 48], BF16, name=f"xp{i}") for i in range(4)]
        dtb_t = [wpool.tile([128, 1], F32, name=f"dtb{i}") for i in range(4)]
        A_t = [wpool.tile([128, D_STATE], F32, name=f"A{i}") for i in range(4)]
        Dp_t = [wpool.tile([128, 1], F32, name=f"Dp{i}") for i in range(4)]
        outw_t = [wpool.tile([128, C], BF16, name=f"ow{i}") for i in range(4)]
        for i in range(4):
            nc.sync.dma_start(out=convw_t[i][:], in_=convw[i])
            nc.sync.dma_start(out=convb_t[i][:], in_=convb[i])
            nc.sync.dma_start(out=xproj_t[i][:], in_=xprojT[i])
            nc.sync.dma_start(out=dtb_t[i][:], in_=dtb[i])
            nc.sync.dma_start(out=A_t[i][:], in_=AT[i])
            nc.sync.dma_start(out=Dp_t[i][:], in_=Dpt[i])
            nc.sync.dma_start(out=outw_t[i][:], in_=outwT[i])
        dtw_t = wpool.tile([16, D_INNER], BF16, name="dtw_t")
        nc.sync.dma_start(out=dtw_t[:], in_=dtwT[:])
        eps_t = wpool.tile([128, 1], F32, name="eps_t")
        nc.vector.memset(eps_t[:], EPS)
        ones_t = wpool.tile([1, 128], BF16, name="ones_t")
        nc.sync.dma_start(out=ones_t[:], in_=ones1[:])
        oneM_t = wpool.tile([128, 128], BF16, name="oneM_t")
        nc.sync.dma_start(out=oneM_t[:], in_=oneM[:])

        # ---- layernorm (stats via PE broadcast-mean matmuls)
        x_bf = [scr.tile([128, L], BF16, name=f"xbf{i}", tag="lnscr") for i in range(2)]
        sq_bf = [scr.tile([128, L], BF16, name=f"sq{i}", tag="lnscr2") for i in range(2)]
        for i in range(2):
            nc.scalar.activation(out=x_bf[i][:], in_=x_t[i][:], func=AF.Copy)
            nc.scalar.activation(out=sq_bf[i][:], in_=x_t[i][:], func=AF.Square)
        cen = [scr.tile([128, L], F32, name=f"cen{i}", tag="cen") for i in range(2)]
        for nch in range(4):
            sl = slice(nch * 512, (nch + 1) * 512)
            mu_p = ps.tile([128, 512], F32, name="mu_p", tag="mm")
            for i in range(2):
                nc.tensor.matmul(mu_p[:], lhsT=oneM_t[:], rhs=x_bf[i][:, sl],
                                 start=(i == 0), stop=(i == 1))
            for i in range(2):
                nc.vector.scalar_tensor_tensor(
                    out=cen[i][:, sl], in0=mu_p[:], scalar=-1.0, in1=x_t[i][:, sl],
                    op0=ALU.mult, op1=ALU.add)
            ss_p = ps.tile([128, 512], F32, name="ss_p", tag="mm")
            for i in range(2):
                nc.tensor.matmul(ss_p[:], lhsT=oneM_t[:], rhs=sq_bf[i][:, sl],
                                 start=(i == 0), stop=(i == 1))
            # var = E[x^2] - mu^2 >= 0 ; inv = exp(-0.5*ln(var+eps))
            mu2 = sc.tile([128, 512], F32, name="mu2")
            nc.scalar.activation(out=mu2[:], in_=mu_p[:], func=AF.Square)
            var_s = sc.tile([128, 512], F32, name="var_s")
            nc.vector.scalar_tensor_tensor(
                out=var_s[:], in0=mu2[:], scalar=-1.0, in1=ss_p[:],
                op0=ALU.mult, op1=ALU.add)
            lnv = sc.tile([128, 512], F32, name="lnv")
            nc.scalar.activation(out=lnv[:], in_=var_s[:], func=AF.Ln, bias=eps_t[:])
            inv_s = sc.tile([128, 512], F32, name="inv_s")
            nc.scalar.activation(out=inv_s[:], in_=lnv[:], func=AF.Exp, scale=-0.5)
            for i in range(2):
                nc.vector.tensor_tensor(out=cen[i][:, sl], in0=cen[i][:, sl],
                                        in1=inv_s[:], op=ALU.mult)
        x_ln = [scr.tile([128, L], BF16, name=f"xln{i}", tag="lnscr") for i in range(2)]
        for i in range(2):
            nc.scalar.activation(out=x_ln[i][:], in_=cen[i][:], func=AF.Identity,
                                 scale=nwb_t[:, 2 * i:2 * i + 1],
                                 bias=nwb_t[:, 2 * i + 1:2 * i + 2])

        # ---- in-proj -> xa (padded, for conv) and silu(z)
        xa_pad = [big.tile([128, 3 + L], BF16, name=f"xap{i}") for i in range(4)]
        for i in range(4):
            nc.vector.memset(xa_pad[i][:, 0:3], 0.0)
        sz = [big.tile([128, L], BF16, name=f"sz{i}") for i in range(4)]
        for m in range(8):
            for nch in range(4):
                sl = slice(nch * 512, (nch + 1) * 512)
                p = ps.tile([128, 512], F32, name="inp_p", tag="mm")
                for i in range(2):
                    nc.tensor.matmul(p[:], lhsT=inw_t[i][:, m * 128:(m + 1) * 128],
                                     rhs=x_ln[i][:, sl], start=(i == 0), stop=(i == 1))
                if m < 4:
                    nc.scalar.activation(out=xa_pad[m][:, 3 + nch * 512:3 + (nch + 1) * 512],
                                         in_=p[:], func=AF.Copy)
                else:
                    nc.scalar.activation(out=sz[m - 4][:, sl], in_=p[:], func=AF.Silu)

        # ---- depthwise causal conv(4) + silu -> u
        u_t = [big.tile([128, L], BF16, name=f"u{i}") for i in range(4)]
        for i in range(4):
            acc = sc.tile([128, L], BF16, name="acc", tag="convacc")
            nc.vector.tensor_scalar_mul(out=acc[:], in0=xa_pad[i][:, 0:L],
                                        scalar1=convw_t[i][:, 0:1])
            for tap in range(1, 4):
                nc.vector.scalar_tensor_tensor(
                    out=acc[:], in0=xa_pad[i][:, tap:tap + L],
                    scalar=convw_t[i][:, tap:tap + 1], in1=acc[:],
                    op0=ALU.mult, op1=ALU.add)
            nc.scalar.activation(out=u_t[i][:], in_=acc[:], func=AF.Silu,
                                 bias=convb_t[i][:])

        # ---- xproj -> dt_lr, B, C rows
        dtlr_s = big.tile([16, L], BF16, name="dtlr_s")
        Bm_s = big.tile([16, L], BF16, name="Bm_s")
        Cm_s = big.tile([16, L], BF16, name="Cm_s")
        for half in range(2):
            sl = slice(half * 1024, (half + 1) * 1024)
            dbc_p = ps4.tile([48, 1024], F32, name="dbc_p")
            for nch in range(2):
                s2 = slice(half * 1024 + nch * 512, half * 1024 + (nch + 1) * 512)
                for i in range(4):
                    nc.tensor.matmul(dbc_p[:, nch * 512:(nch + 1) * 512],
                                     lhsT=xproj_t[i][:], rhs=u_t[i][:, s2],
                                     start=(i == 0), stop=(i == 3))
            nc.scalar.activation(out=dtlr_s[:, sl], in_=dbc_p[0:16, :], func=AF.Copy)
            nc.scalar.activation(out=Bm_s[:, sl], in_=dbc_p[16:32, :], func=AF.Copy)
            nc.scalar.activation(out=Cm_s[:, sl], in_=dbc_p[32:48, :], func=AF.Copy)

        # ---- dt = softplus(dt_w @ dt_lr + dt_b)  [512, L] bf16
        dt_t = [big.tile([128, L], BF16, name=f"dt{i}") for i in range(4)]
        dtu_t = [big.tile([128, L], BF16, name=f"dtu{i}") for i in range(4)]
        for m in range(4):
            for nch in range(4):
                sl = slice(nch * 512, (nch + 1) * 512)
                p = ps.tile([128, 512], F32, name="dt_p", tag="mm")
                nc.tensor.matmul(p[:], lhsT=dtw_t[:, m * 128:(m + 1) * 128],
                                 rhs=dtlr_s[:, sl], start=True, stop=True)
                nc.scalar.activation(out=dt_t[m][:, sl], in_=p[:], func=AF.Softplus,
                                     bias=dtb_t[m][:])
        for m in range(4):
            nc.vector.tensor_tensor(out=dtu_t[m][:], in0=dt_t[m][:], in1=u_t[m][:],
                                    op=ALU.mult)

        # ---- selective scan, t-tiled; y accumulated over the 16 states
        hlast = [wpool.tile([128, D_STATE], F32, name=f"hl{i}") for i in range(4)]
        for i in range(4):
            nc.vector.memset(hlast[i][:], 0.0)
        y_acc = [big.tile([128, L], BF16, name=f"ya{i}") for i in range(4)]
        B_bc = [bc.tile([128, TT], BF16, name=f"Bb{j}") for j in range(8)]
        C_bc = [bc.tile([128, TT], BF16, name=f"Cb{j}") for j in range(8)]
        for ttg in range(2 * (L // TT)):
            tt, ng = divmod(ttg, 2)
            tsl = slice(tt * TT, (tt + 1) * TT)
            for j in range(8):
                n = ng * 8 + j
                # stage row n at partition 0 (DMA moves across partitions)
                brow = sc.tile([1, TT], BF16, name="brow", tag="brow")
                nc.sync.dma_start(out=brow[:], in_=Bm_s[n:n + 1, tsl])
                pb = ps.tile([128, TT], F32, name="pb", tag="mm")
                nc.tensor.matmul(pb[:], lhsT=ones_t[:], rhs=brow[:],
                                 start=True, stop=True)
                nc.scalar.activation(out=B_bc[j][:], in_=pb[:], func=AF.Copy)
                crow = sc.tile([1, TT], BF16, name="crow", tag="crow")
                nc.sync.dma_start(out=crow[:], in_=Cm_s[n:n + 1, tsl])
                pc = ps.tile([128, TT], F32, name="pc", tag="mm")
                nc.tensor.matmul(pc[:], lhsT=ones_t[:], rhs=crow[:],
                                 start=True, stop=True)
                nc.scalar.activation(out=C_bc[j][:], in_=pc[:], func=AF.Copy)
            for dblk in range(4):
                for j in range(8):
                    n = ng * 8 + j
                    dA = sc.tile([128, TT], BF16, name="dA", tag="dA")
                    nc.scalar.activation(out=dA[:], in_=dt_t[dblk][:, tsl],
                                         func=AF.Exp, scale=A_t[dblk][:, n:n + 1])
                    dBu = sc.tile([128, TT], BF16, name="dBu", tag="dBu")
                    nc.vector.scalar_tensor_tensor(
                        out=dBu[:], in0=dtu_t[dblk][:, tsl], scalar=1.0,
                        in1=B_bc[j][:], op0=ALU.mult, op1=ALU.mult)
                    h = sc.tile([128, TT], BF16, name="h", tag="h")
                    nc.vector.tensor_tensor_scan(
                        out=h[:], data0=dA[:], data1=dBu[:],
                        initial=hlast[dblk][:, n:n + 1], op0=ALU.mult, op1=ALU.add)
                    nc.vector.tensor_copy(out=hlast[dblk][:, n:n + 1], in_=h[:, TT - 1:TT])
                    if n == 0:
                        nc.vector.scalar_tensor_tensor(
                            out=y_acc[dblk][:, tsl], in0=h[:], scalar=1.0,
                            in1=C_bc[j][:], op0=ALU.mult, op1=ALU.mult)
                    else:
                        yn = sc.tile([128, TT], BF16, name="yn", tag="yn")
                        nc.vector.scalar_tensor_tensor(
                            out=yn[:], in0=h[:], scalar=1.0,
                            in1=C_bc[j][:], op0=ALU.mult, op1=ALU.mult)
                        nc.vector.tensor_tensor(out=y_acc[dblk][:, tsl],
                                                in0=y_acc[dblk][:, tsl],
                                                in1=yn[:], op=ALU.add)

        # ---- y = (y_acc + Dp*u) * silu(z); out = out_w @ y + x
        yg = [big.tile([128, L], BF16, name=f"yg{i}") for i in range(4)]
        for m in range(4):
            nc.vector.scalar_tensor_tensor(
                out=yg[m][:], in0=u_t[m][:], scalar=Dp_t[m][:], in1=y_acc[m][:],
                op0=ALU.mult, op1=ALU.add)
            nc.vector.tensor_tensor(out=yg[m][:], in0=yg[m][:], in1=sz[m][:],
                                    op=ALU.mult)
        for m in range(2):
            for nch in range(4):
                sl = slice(nch * 512, (nch + 1) * 512)
                p = ps.tile([128, 512], F32, name="out_p", tag="mm")
                for i in range(4):
                    nc.tensor.matmul(p[:], lhsT=outw_t[i][:, m * 128:(m + 1) * 128],
                                     rhs=yg[i][:, sl], start=(i == 0), stop=(i == 3))
                o = sc.tile([128, 512], F32, name="o", tag="outsc")
                nc.vector.scalar_tensor_tensor(
                    out=o[:], in0=p[:], scalar=1.0, in1=x_t[m][:, sl],
                    op0=ALU.mult, op1=ALU.add)
                nc.sync.dma_start(out=outT[m, :, sl], in_=o[:])
        stack.close()
    nc.compile()
    return nc


def prep_phase1_inputs(inputs, xs, core):
    b, k = divmod(core, 4)
    A = -np.exp(inputs['A_log'][k]).astype(np.float32)          # [512, 16]
    return {
        "xT": np.ascontiguousarray(xs[b, k]).reshape(2, 128, L).astype(np.float32),
        "nwb": np.stack([inputs['norm_w'][k].reshape(2, 128),
                         inputs['norm_b'][k].reshape(2, 128)], 2).astype(np.float32),
        "inwT": _bf16(inputs['in_w'][k].T.reshape(2, 128, 2 * D_INNER)),
        "convw": inputs['conv_w'][k][:, 0, :].reshape(4, 128, D_CONV).astype(np.float32),
        "convb": inputs['conv_b'][k].reshape(4, 128, 1).astype(np.float32),
        "xprojT": _bf16(inputs['xproj_w'][k].T.reshape(4, 128, 48)),
        "dtwT": _bf16(inputs['dt_w'][k].T),
        "dtb": inputs['dt_b'][k].reshape(4, 128, 1).astype(np.float32),
        "AT": A.reshape(4, 128, D_STATE),
        "Dpt": inputs['Dp'][k].reshape(4, 128, 1).astype(np.float32),
        "outwT": _bf16(inputs['out_w'][k].T.reshape(4, 128, C)),
        "ones1": _bf16(np.ones((1, 128))),
        "oneM": _bf16(np.full((128, 128), 1.0 / 256.0)),
    }


def run_phase1_bass(inputs, xs):
    if "p1" not in _cache:
        _cache["p1"] = build_phase1()
    nc = _cache["p1"]
    ins = [prep_phase1_inputs(inputs, xs, core) for core in range(8)]
    res = run_bass_kernel_spmd(nc, ins, list(range(8)))
    return [res.results[c]["outT"].reshape(C, L) for c in range(8)]


# ---------------------------------------------------------------------------
# numpy reference fallback (kept for testing)
# ---------------------------------------------------------------------------

def _sigmoid(v):
    return 1.0 / (1.0 + np.exp(-v))


def mamba_block_np(xT, nw, nb, in_w, conv_w, conv_b, xproj_w, dt_w, dt_b,
                   A_log, Dp, out_w):
    x = xT.T
    mu = x.mean(-1, keepdims=True)
    var = ((x - mu) ** 2).mean(-1, keepdims=True)
    h = (x - mu) / np.sqrt(var + EPS) * nw + nb
    xz = h @ in_w.T
    xa, z = xz[:, :D_INNER], xz[:, D_INNER:]
    xa_t = xa.T
    w = conv_w[:, 0, :]
    pad = np.pad(xa_t, ((0, 0), (D_CONV - 1, 0)))
    conv = sum(pad[:, i:i + L] * w[:, i:i + 1] for i in range(D_CONV))
    u_t = conv + conv_b[:, None]
    u_t = u_t * _sigmoid(u_t)
    u = u_t.T
    dbc = u @ xproj_w.T
    dt_lr = dbc[:, :DT_RANK]
    Bm = dbc[:, DT_RANK:DT_RANK + D_STATE]
    Cm = dbc[:, DT_RANK + D_STATE:]
    vv = dt_lr @ dt_w.T + dt_b
    dt = np.log1p(np.exp(-np.abs(vv))) + np.maximum(vv, 0)
    A = -np.exp(A_log)
    dA = np.exp(dt[:, :, None] * A[None])
    dBu = dt[:, :, None] * Bm[:, None, :] * u[:, :, None]
    hs = np.zeros((D_INNER, D_STATE), np.float32)
    ys = np.zeros((L, D_INNER), np.float32)
    for t in range(L):
        hs = dA[t] * hs + dBu[t]
        ys[t] = (hs * Cm[t][None, :]).sum(-1)
    y = ys + Dp * u
    y = y * (z * _sigmoid(z))
    mo = y @ out_w.T
    return xT + mo.T


def run_phase1_np(inputs, xs):
    outs = []
    for core in range(8):
        b, k = divmod(core, 4)
        outs.append(mamba_block_np(
            np.ascontiguousarray(xs[b, k]), inputs['norm_w'][k], inputs['norm_b'][k],
            inputs['in_w'][k], inputs['conv_w'][k], inputs['conv_b'][k],
            inputs['xproj_w'][k], inputs['dt_w'][k], inputs['dt_b'][k],
            inputs['A_log'][k], inputs['Dp'][k], inputs['out_w'][k]))
    return outs


# ---------------------------------------------------------------------------
# top level
# ---------------------------------------------------------------------------

def kernel(**inputs):
    inputs = {k: np.asarray(v, np.float32) if np.asarray(v).dtype == np.float32
              else np.asarray(v) for k, v in inputs.items()}
    xs = scan_jego_np(inputs['feat0'], inputs['feat1'])  # [B,4,C,L]
    p1 = run_phase1(inputs, xs)
    ys = np.stack([np.stack(p1[4 * b:4 * b + 4], 0) for b in range(B)], 0)
    d0, d1 = merge_jego_np(ys, H8, W8)
    Dfull = np.concatenate([d0, d1], 0)
    desc = run_phase2(Dfull, inputs['glu_w'], inputs['glu_b'])
    dd0, dd1 = desc[:B], desc[B:]
    return np.stack([dd0.reshape(B, C, -1), dd1.reshape(B, C, -1)], 0).astype(np.float32)


def run_phase1(inputs, xs):
    return run_phase1_np(inputs, xs)



# revision 29
# speedup vs baseline: 10956.0783x; 10956.0783x over previous
"""Trainium2 Bass kernel for nn_JointMamba: 4-direction Mamba scan + GLU conv.

Sharding: phase 1 runs the 8 independent (batch, direction) Mamba blocks one
per NeuronCore; phase 2 reshards the merged feature maps over (image, row-half)
and runs the 3x3 GLU conv, one shard per core. Host does only permutations /
layout prep (scan_jego / merge_jego are pure index shuffles).
"""
import sys
import numpy as np

try:
    import concourse.bass as bass  # noqa: F401
except ImportError:
    sys.path.insert(0, "/opt/trn_rl_repo")

import concourse.bass as bass
import concourse.bacc as bacc
import concourse.mybir as mybir
from concourse.bass_utils import run_bass_kernel_spmd
from concourse import tile

F32 = mybir.dt.float32
BF16 = mybir.dt.bfloat16
ALU = mybir.AluOpType
AF = mybir.ActivationFunctionType

B, C, H8, W8 = 2, 256, 64, 64
D_INNER, D_STATE, D_CONV, DT_RANK = 512, 16, 4, 16
L = (H8 // 2) * W8  # 2048
EPS = 1e-5

_cache = {}


def _bf16(x):
    import ml_dtypes
    return np.asarray(x, dtype=ml_dtypes.bfloat16)


# ---------------------------------------------------------------------------
# host-side permutations (pure data movement)
# ---------------------------------------------------------------------------

def scan_jego_np(d0, d1):
    d2w = np.concatenate([d0, d1], 3)
    d2h = np.concatenate([d0, d1], 2)
    b, c = d0.shape[:2]
    x0 = d2w[:, :, ::2, ::2].reshape(b, c, -1)
    x1 = np.swapaxes(d2h, 2, 3)[:, :, 1::2, 1::2].reshape(b, c, -1)
    x2 = d2w[:, :, ::2, 1::2].reshape(b, c, -1)[:, :, ::-1]
    x3 = np.swapaxes(d2h, 2, 3)[:, :, ::2, 1::2].reshape(b, c, -1)[:, :, ::-1]
    return np.stack([x0, x1, x2, x3], 1)  # [B,4,C,L]


def merge_jego_np(ys, ori_h, ori_w):
    b, k, c, Lx = ys.shape
    H, W = ori_h // 2, ori_w // 2
    y2w = np.zeros((b, c, ori_h, 2 * ori_w), ys.dtype)
    y2h = np.zeros((b, c, 2 * ori_h, ori_w), ys.dtype)
    y2w[:, :, ::2, ::2] = ys[:, 0].reshape(b, c, H, 2 * W)
    y2h[:, :, 1::2, 1::2] = np.swapaxes(ys[:, 1].reshape(b, c, W, 2 * H), 2, 3)
    y2w[:, :, ::2, 1::2] = ys[:, 2][:, :, ::-1].reshape(b, c, H, 2 * W)
    y2h[:, :, 1::2, ::2] = np.swapaxes(ys[:, 3][:, :, ::-1].reshape(b, c, W, 2 * H), 2, 3)
    d0w, d1w = np.split(y2w, 2, axis=3)
    d0h, d1h = np.split(y2h, 2, axis=2)
    return d0w + d0h, d1w + d1h


# ---------------------------------------------------------------------------
# phase 2: 3x3 conv + GLU, sharded over (image, row-half)
# ---------------------------------------------------------------------------

def build_phase2():
    """Per core: dpad [2,128,34*66] bf16, wc [9,2,128,512] bf16 (lhsT per tap),
    bias [128,4] f32.  Output o [2,128,2048] f32 (= [256, 32, 64] GLU'd rows)."""
    nc = bacc.Bacc("TRN2", target_bir_lowering=False, debug=False, num_devices=8)
    dpad = nc.dram_tensor("dpad", [2, 128, 34 * 66], BF16, kind="ExternalInput")
    wc = nc.dram_tensor("wc", [9, 2, 128, 512], BF16, kind="ExternalInput")
    bias = nc.dram_tensor("bias", [128, 4], F32, kind="ExternalInput")
    out = nc.dram_tensor("o", [2, 128, 2048], F32, kind="ExternalOutput")

    with tile.TileContext(nc) as tc:
        with tc.tile_pool(name="cw", bufs=1) as cw, \
             tc.tile_pool(name="cd", bufs=1) as cd, \
             tc.tile_pool(name="cpsum", bufs=2, space="PSUM") as cpsum, \
             tc.tile_pool(name="cact", bufs=3) as cact:
            dbf = []
            for kc in range(2):
                d = cd.tile([128, 34 * 66], BF16, name=f"d{kc}")
                nc.sync.dma_start(out=d[:], in_=dpad[kc])
                dbf.append(d)
            wt = []
            for tap in range(9):
                row_w = []
                for kc in range(2):
                    w_ = cw.tile([128, 512], BF16, name=f"w{tap}_{kc}")
                    nc.sync.dma_start(out=w_[:], in_=wc[tap, kc])
                    row_w.append(w_)
                wt.append(row_w)
            bias_t = cw.tile([128, 4], F32, name="bias_t")
            nc.sync.dma_start(out=bias_t[:], in_=bias[:])

            for rg in range(4):  # row groups of 8 output rows
                ps = []
                for m in range(4):  # co tiles of 128
                    p = cpsum.tile([128, 512], F32, name=f"ps{m}")
                    ps.append(p)
                    for ti, tap in enumerate(range(9)):
                        dy, dx = divmod(tap, 3)
                        for kc in range(2):
                            rhs_bf = dbf[kc][:, (rg * 8 + dy) * 66 + dx:]
                            rhs_bf = bass.AP(rhs_bf.tensor, rhs_bf.offset,
                                             [rhs_bf.ap[0], [66, 8], [1, 64]])
                            nc.tensor.matmul(
                                p[:], lhsT=wt[tap][kc][:, m * 128:(m + 1) * 128],
                                rhs=rhs_bf, start=(ti == 0 and kc == 0),
                                stop=(tap == 8 and kc == 1))
                # GLU: a = ps[0..1], g = ps[2..3]
                for m in range(2):
                    sg = cact.tile([128, 512], F32, name="sg")
                    nc.scalar.activation(out=sg[:], in_=ps[2 + m][:],
                                         func=AF.Sigmoid, bias=bias_t[:, 2 + m:3 + m])
                    av = cact.tile([128, 512], F32, name="av")
                    nc.scalar.activation(out=av[:], in_=ps[m][:],
                                         func=AF.Identity, bias=bias_t[:, m:m + 1])
                    og = cact.tile([128, 512], F32, name="og")
                    nc.vector.tensor_tensor(out=og[:], in0=av[:], in1=sg[:], op=ALU.mult)
                    nc.sync.dma_start(out=out[m, :, rg * 512:(rg + 1) * 512], in_=og[:])
    nc.compile()
    return nc


def prep_phase2_weights(glu_w, glu_b):
    # wc[tap, kc, ci, co] = glu_w[co, kc*128+ci, dy, dx]
    w = np.transpose(glu_w, (2, 3, 1, 0)).reshape(9, 2, 128, 512)
    bias = glu_b.reshape(4, 128).T.copy()  # [128, 4] per-partition
    return _bf16(w), bias


def run_phase2(Dfull, glu_w, glu_b):
    """Dfull [4, 256, 64, 64] -> [4, 256, 64, 64] after conv+GLU."""
    if "p2" not in _cache:
        _cache["p2"] = build_phase2()
    nc = _cache["p2"]
    w_hi, bias = prep_phase2_weights(glu_w, glu_b)
    Dpad = np.pad(Dfull, ((0, 0), (0, 0), (1, 1), (1, 1)))
    ins = []
    for core in range(8):
        img, half = divmod(core, 2)
        r0 = half * 32
        dslice = Dpad[img, :, r0:r0 + 34, :].reshape(2, 128, 34 * 66)
        ins.append({"dpad": _bf16(dslice), "wc": w_hi, "bias": bias})
    res = run_bass_kernel_spmd(nc, ins, list(range(8)))
    out = np.zeros((4, 256, 64, 64), np.float32)
    for core in range(8):
        img, half = divmod(core, 2)
        o = res.results[core]["o"].reshape(256, 32, 64)
        out[img, :, half * 32:half * 32 + 32, :] = o
    return out


# ---------------------------------------------------------------------------
# phase 1: per-(b,k) Mamba block on one core
# layout: feature-major ([channel, t]) throughout; selective scan uses the
# native DVE TensorTensorScan along the free (t) axis, 16 state rows per
# d-block handled as independent [128, T] sweeps.
# ---------------------------------------------------------------------------

HC_GP8 = 4    # of every 8 hC multiplies, this many go to gpsimd
DBU_GP8 = 4   # of every 8 dBu multiplies, this many go to gpsimd
HLAST_GP = True  # hlast column copies on gpsimd


def build_phase1():
    nc = bacc.Bacc("TRN2", target_bir_lowering=False, debug=False, num_devices=8)
    xT = nc.dram_tensor("xT", [2, 128, L], F32, kind="ExternalInput")
    nwb = nc.dram_tensor("nwb", [2, 128, 2], F32, kind="ExternalInput")      # nw, nb
    inwT = nc.dram_tensor("inwT", [2, 128, 2 * D_INNER], BF16, kind="ExternalInput")
    convw = nc.dram_tensor("convw", [4, 128, D_CONV], F32, kind="ExternalInput")
    convb = nc.dram_tensor("convb", [4, 128, 1], F32, kind="ExternalInput")
    xprojT = nc.dram_tensor("xprojT", [4, 128, 48], BF16, kind="ExternalInput")
    dtwT = nc.dram_tensor("dtwT", [16, D_INNER], BF16, kind="ExternalInput")
    dtb = nc.dram_tensor("dtb", [4, 128, 1], F32, kind="ExternalInput")
    AT = nc.dram_tensor("AT", [4, 128, D_STATE], F32, kind="ExternalInput")
    Dpt = nc.dram_tensor("Dpt", [4, 128, 1], F32, kind="ExternalInput")
    outwT = nc.dram_tensor("outwT", [4, 128, C], BF16, kind="ExternalInput")
    identT = nc.dram_tensor("identT", [128, 128], BF16, kind="ExternalInput")
    inb = nc.dram_tensor("inb", [128, 8], F32, kind="ExternalInput")  # in_w @ nb
    oneM = nc.dram_tensor("oneM", [128, 128], BF16, kind="ExternalInput")    # 1/256
    BmD = nc.dram_tensor("BmD", [16, L], BF16, kind="Internal")
    CmD = nc.dram_tensor("CmD", [16, L], BF16, kind="Internal")
    outT = nc.dram_tensor("outT", [2, 128, L], F32, kind="ExternalOutput")

    TT = 512  # t-tile for the scan stage (PSUM-bank bound)
    with tile.TileContext(nc) as tc:
        import contextlib
        stack = contextlib.ExitStack()
        wpool = stack.enter_context(tc.tile_pool(name="wpool", bufs=1))
        big = stack.enter_context(tc.tile_pool(name="big", bufs=1))
        ps = stack.enter_context(tc.tile_pool(name="ps", bufs=4, space="PSUM"))
        psY = stack.enter_context(tc.tile_pool(name="psY", bufs=1, space="PSUM"))
        scr = stack.enter_context(tc.tile_pool(name="scr", bufs=2))
        bc = stack.enter_context(tc.tile_pool(name="bc", bufs=1))
        sc = stack.enter_context(tc.tile_pool(name="sc", bufs=3))
        cvp = stack.enter_context(tc.tile_pool(name="cvp", bufs=2))

        # ---- load inputs
        x_t = [wpool.tile([128, L], F32, name=f"x{i}") for i in range(2)]
        for i in range(2):
            nc.sync.dma_start(out=x_t[i][:], in_=xT[i])
        nwb_t = wpool.tile([128, 4], F32, name="nwb_t")
        for i in range(2):
            nc.sync.dma_start(out=nwb_t[:, 2 * i:2 * i + 2], in_=nwb[i])
        inw_t = [wpool.tile([128, 2 * D_INNER], BF16, name=f"inw{i}") for i in range(2)]
        for i in range(2):
            nc.sync.dma_start(out=inw_t[i][:], in_=inwT[i])
        convw_t = [wpool.tile([128, D_CONV], F32, name=f"cw{i}") for i in range(4)]
        convb_t = [wpool.tile([128, 1], F32, name=f"cb{i}") for i in range(4)]
        xproj_t = [wpool.tile([128, 48], BF16, name=f"xp{i}") for i in range(4)]
        dtb_t = [wpool.tile([128, 1], F32, name=f"dtb{i}") for i in range(4)]
        A_t = [wpool.tile([128, D_STATE], F32, name=f"A{i}") for i in range(4)]
        Dp_t = [wpool.tile([128, 1], F32, name=f"Dp{i}") for i in range(4)]
        outw_t = [wpool.tile([128, C], BF16, name=f"ow{i}") for i in range(4)]
        for i in range(4):
            nc.sync.dma_start(out=convw_t[i][:], in_=convw[i])
            nc.sync.dma_start(out=convb_t[i][:], in_=convb[i])
            nc.sync.dma_start(out=xproj_t[i][:], in_=xprojT[i])
            nc.sync.dma_start(out=dtb_t[i][:], in_=dtb[i])
            nc.sync.dma_start(out=A_t[i][:], in_=AT[i])
            nc.sync.dma_start(out=Dp_t[i][:], in_=Dpt[i])
            nc.sync.dma_start(out=outw_t[i][:], in_=outwT[i])
        dtw_t = wpool.tile([16, D_INNER], BF16, name="dtw_t")
        nc.sync.dma_start(out=dtw_t[:], in_=dtwT[:])
        eps_t = wpool.tile([128, 1], F32, name="eps_t")
        nc.vector.memset(eps_t[:], EPS)
        ident_t = wpool.tile([128, 128], BF16, name="ident_t")
        nc.sync.dma_start(out=ident_t[:], in_=identT[:])
        inb_t = wpool.tile([128, 8], F32, name="inb_t")
        nc.sync.dma_start(out=inb_t[:], in_=inb[:])
        oneM_t = wpool.tile([128, 128], BF16, name="oneM_t")
        nc.sync.dma_start(out=oneM_t[:], in_=oneM[:])

        # ---- layernorm fused with in-proj, chunked over t.
        # nb is folded into the in-proj bias (inb) on the host; x_ln = (x-mu)*inv*nw.
        xa_pad = [big.tile([128, 3 + L], BF16, name=f"xap{i}") for i in range(4)]
        for i in range(4):
            nc.vector.memset(xa_pad[i][:, 0:3], 0.0)
        sz = [big.tile([128, L], BF16, name=f"sz{i}") for i in range(4)]
        for nch in range(4):
            sl = slice(nch * 512, (nch + 1) * 512)
            x_bf = [scr.tile([128, 512], BF16, name=f"xbf{i}", tag=f"xbf{i}")
                    for i in range(2)]
            sq_bf = [scr.tile([128, 512], BF16, name=f"sq{i}", tag=f"sq{i}")
                     for i in range(2)]
            for i in range(2):
                nc.scalar.activation(out=x_bf[i][:], in_=x_t[i][:, sl], func=AF.Copy)
                nc.scalar.activation(out=sq_bf[i][:], in_=x_t[i][:, sl], func=AF.Square)
            mu_p = ps.tile([128, 512], F32, name="mu_p", tag="mm")
            for i in range(2):
                nc.tensor.matmul(mu_p[:], lhsT=oneM_t[:], rhs=x_bf[i][:],
                                 start=(i == 0), stop=(i == 1))
            ss_p = ps.tile([128, 512], F32, name="ss_p", tag="mm")
            for i in range(2):
                nc.tensor.matmul(ss_p[:], lhsT=oneM_t[:], rhs=sq_bf[i][:],
                                 start=(i == 0), stop=(i == 1))
            # var = E[x^2] - mu^2 >= 0 ; inv = exp(-0.5*ln(var+eps))
            mu2 = scr.tile([128, 512], F32, name="mu2", tag="mu2")
            nc.scalar.activation(out=mu2[:], in_=mu_p[:], func=AF.Square)
            var_s = scr.tile([128, 512], F32, name="var_s", tag="var_s")
            nc.vector.scalar_tensor_tensor(
                out=var_s[:], in0=mu2[:], scalar=-1.0, in1=ss_p[:],
                op0=ALU.mult, op1=ALU.add)
            lnv = scr.tile([128, 512], F32, name="lnv", tag="lnv")
            nc.scalar.activation(out=lnv[:], in_=var_s[:], func=AF.Ln, bias=eps_t[:])
            inv_b = scr.tile([128, 512], BF16, name="inv_b", tag="inv_b")
            nc.scalar.activation(out=inv_b[:], in_=lnv[:], func=AF.Exp, scale=-0.5)
            x_ln = [scr.tile([128, 512], BF16, name=f"xln{i}", tag=f"xln{i}")
                    for i in range(2)]
            for i in range(2):
                cen = scr.tile([128, 512], BF16, name="cen", tag="cen")
                nc.vector.scalar_tensor_tensor(
                    out=cen[:], in0=mu_p[:], scalar=-1.0, in1=x_t[i][:, sl],
                    op0=ALU.mult, op1=ALU.add)
                nc.vector.scalar_tensor_tensor(
                    out=x_ln[i][:], in0=cen[:], scalar=nwb_t[:, 2 * i:2 * i + 1],
                    in1=inv_b[:], op0=ALU.mult, op1=ALU.mult)
            for m in range(8):
                p = ps.tile([128, 512], F32, name="inp_p", tag="mm")
                for i in range(2):
                    nc.tensor.matmul(p[:], lhsT=inw_t[i][:, m * 128:(m + 1) * 128],
                                     rhs=x_ln[i][:], start=(i == 0), stop=(i == 1))
                if m < 4:
                    nc.scalar.activation(out=xa_pad[m][:, 3 + nch * 512:3 + (nch + 1) * 512],
                                         in_=p[:], func=AF.Identity, bias=inb_t[:, m:m + 1])
                else:
                    # raw z for now; silu applied in one batched pass later
                    # (silu lives in a different ACT table set than exp/ln)
                    nc.scalar.activation(out=sz[m - 4][:, sl], in_=p[:],
                                         func=AF.Identity, bias=inb_t[:, m:m + 1])

        # ---- depthwise causal conv(4) + silu -> u, chunked over t.
        # tap-multiplies via tensor_scalar (4x mode); adds via tensor_tensor (2x).
        u_t = [big.tile([128, L], BF16, name=f"u{i}") for i in range(4)]
        for i in range(4):
            for cch in range(4):
                base = cch * 512  # xa_pad column base (pad offset 3 built in)
                t0 = cvp.tile([128, 512], BF16, name="t0", tag="convt0")
                nc.vector.tensor_scalar_mul(out=t0[:], in0=xa_pad[i][:, base:base + 512],
                                            scalar1=convw_t[i][:, 0:1])
                t1 = cvp.tile([128, 512], BF16, name="t1", tag="convt1")
                nc.vector.tensor_scalar_mul(out=t1[:], in0=xa_pad[i][:, base + 1:base + 513],
                                            scalar1=convw_t[i][:, 1:2])
                nc.vector.tensor_tensor(out=t0[:], in0=t0[:], in1=t1[:], op=ALU.add)
                nc.vector.tensor_scalar_mul(out=t1[:], in0=xa_pad[i][:, base + 2:base + 514],
                                            scalar1=convw_t[i][:, 2:3])
                t2 = cvp.tile([128, 512], BF16, name="t2", tag="convt2")
                nc.vector.tensor_scalar_mul(out=t2[:], in0=xa_pad[i][:, base + 3:base + 515],
                                            scalar1=convw_t[i][:, 3:4])
                nc.vector.tensor_tensor(out=t1[:], in0=t1[:], in1=t2[:], op=ALU.add)
                nc.vector.tensor_tensor(out=u_t[i][:, cch * 512:(cch + 1) * 512],
                                        in0=t0[:], in1=t1[:], op=ALU.add)

        # batched silu pass (one ACT table switch in, one out):
        # u = silu(u + convb), sz = silu(sz)
        for i in range(4):
            nc.scalar.activation(out=u_t[i][:], in_=u_t[i][:], func=AF.Silu,
                                 bias=convb_t[i][:])
            nc.scalar.activation(out=sz[i][:], in_=sz[i][:], func=AF.Silu)

        # ---- xproj -> dt_lr (partitions 0:16), B (16:32), C (32:48)
        dbc_all = big.tile([48, L], BF16, name="dbc_all")
        for nch in range(4):
            sl = slice(nch * 512, (nch + 1) * 512)
            dbc_p = ps.tile([48, 512], F32, name="dbc_p", tag="mm")
            for i in range(4):
                nc.tensor.matmul(dbc_p[:], lhsT=xproj_t[i][:], rhs=u_t[i][:, sl],
                                 start=(i == 0), stop=(i == 3))
            nc.scalar.activation(out=dbc_all[:, sl], in_=dbc_p[:], func=AF.Copy)
            # bounce B/C rows through DRAM so they can be partition-broadcast
            # back by pure DMA (stride-0 partition APs) during the scan.
            nc.sync.dma_start(out=BmD[:, sl], in_=dbc_all[16:32, sl])
            nc.sync.dma_start(out=CmD[:, sl], in_=dbc_all[32:48, sl])

        # ---- dt = softplus(dt_w @ dt_lr + dt_b)  [512, L] bf16
        dt_t = [big.tile([128, L], BF16, name=f"dt{i}") for i in range(4)]
        dtu_t = [big.tile([128, L], BF16, name=f"dtu{i}") for i in range(4)]
        for m in range(4):
            for nch in range(4):
                sl = slice(nch * 512, (nch + 1) * 512)
                p = ps.tile([128, 512], F32, name="dt_p", tag="mm")
                nc.tensor.matmul(p[:], lhsT=dtw_t[:, m * 128:(m + 1) * 128],
                                 rhs=dbc_all[0:16, sl], start=True, stop=True)
                # softplus(v) = ln(1 + exp(v)) (no softplus in the ACT tables;
                # v stays small here so exp cannot overflow)
                e_s = scr.tile([128, 512], F32, name="e_s", tag="e_s")
                nc.scalar.activation(out=e_s[:], in_=p[:], func=AF.Exp,
                                     bias=dtb_t[m][:])
                nc.scalar.activation(out=dt_t[m][:, sl], in_=e_s[:], func=AF.Ln,
                                     bias=1.0)
        for m in range(4):
            nc.vector.tensor_tensor(out=dtu_t[m][:], in0=dt_t[m][:], in1=u_t[m][:],
                                    op=ALU.mult)

        # ---- selective scan (chunk-major).  Per t-chunk of 512:
        #  - one-hot PE matmuls broadcast B/C rows to 128 partitions,
        #  - Act computes dA = exp(A_n * dt), DVE computes dBu / scan / hC,
        #  - identity matmuls accumulate sum_n C_n*h_n in PSUM (psY, 4 banks).
        hlast = [wpool.tile([128, D_STATE], F32, name=f"hl{i}") for i in range(4)]
        for i in range(4):
            nc.vector.memset(hlast[i][:], 0.0)
        yg = [big.tile([128, L], BF16, name=f"yg{i}") for i in range(4)]
        for cch in range(L // TT):
            tsl = slice(cch * TT, (cch + 1) * TT)
            psy = [psY.tile([128, TT], F32, name=f"y{dblk}", tag=f"y{dblk}")
                   for dblk in range(4)]
            for ng in range(2):
                B_bc = []
                C_bc = []
                for j in range(8):
                    n = ng * 8 + j
                    bb = bc.tile([128, TT], BF16, name=f"bb{j}", tag=f"bb{j}")
                    nc.sync.dma_start(out=bb[:], in_=bass.AP(
                        BmD, n * L + cch * TT, [[0, 128], [1, TT]]))
                    B_bc.append(bb)
                    cb = bc.tile([128, TT], BF16, name=f"cb{j}", tag=f"cb{j}")
                    nc.sync.dma_start(out=cb[:], in_=bass.AP(
                        CmD, n * L + cch * TT, [[0, 128], [1, TT]]))
                    C_bc.append(cb)
                for dblk in range(4):
                    for j in range(8):
                        n = ng * 8 + j
                        un = dblk * 16 + n
                        dA = sc.tile([128, TT], BF16, name="dA", tag="dA")
                        nc.scalar.activation(out=dA[:], in_=dt_t[dblk][:, tsl],
                                             func=AF.Exp, scale=A_t[dblk][:, n:n + 1])
                        dBu = sc.tile([128, TT], BF16, name="dBu", tag="dBu")
                        dbu_eng = nc.gpsimd if (un % 8) < DBU_GP8 else nc.vector
                        dbu_eng.tensor_tensor(
                            out=dBu[:], in0=dtu_t[dblk][:, tsl],
                            in1=B_bc[j][:], op=ALU.mult)
                        h = sc.tile([128, TT], BF16, name="h", tag="h")
                        nc.vector.tensor_tensor_scan(
                            out=h[:], data0=dA[:], data1=dBu[:],
                            initial=hlast[dblk][:, n:n + 1], op0=ALU.mult, op1=ALU.add)
                        hl_eng = nc.gpsimd if HLAST_GP else nc.vector
                        hl_eng.tensor_copy(out=hlast[dblk][:, n:n + 1],
                                           in_=h[:, TT - 1:TT])
                        hc = sc.tile([128, TT], BF16, name="hc", tag="hc")
                        hc_eng = nc.gpsimd if (un % 8) >= 8 - HC_GP8 else nc.vector
                        hc_eng.tensor_tensor(out=hc[:], in0=h[:],
                                             in1=C_bc[j][:], op=ALU.mult)
                        nc.tensor.matmul(psy[dblk][:], lhsT=ident_t[:], rhs=hc[:],
                                         start=(n == 0), stop=(n == 15))
            # y + Dp*u straight out of PSUM (gate with silu(z) later)
            for dblk in range(4):
                nc.vector.scalar_tensor_tensor(
                    out=yg[dblk][:, tsl], in0=u_t[dblk][:, tsl],
                    scalar=Dp_t[dblk][:], in1=psy[dblk][:],
                    op0=ALU.mult, op1=ALU.add)

        # ---- y = (y + Dp*u) * silu(z); out = out_w @ y + x
        for m in range(4):
            nc.vector.tensor_tensor(out=yg[m][:], in0=yg[m][:], in1=sz[m][:],
                                    op=ALU.mult)
        for m in range(2):
            for nch in range(4):
                sl = slice(nch * 512, (nch + 1) * 512)
                p = ps.tile([128, 512], F32, name="out_p", tag="mm")
                for i in range(4):
                    nc.tensor.matmul(p[:], lhsT=outw_t[i][:, m * 128:(m + 1) * 128],
                                     rhs=yg[i][:, sl], start=(i == 0), stop=(i == 3))
                o = sc.tile([128, 512], F32, name="o", tag="outsc")
                nc.vector.scalar_tensor_tensor(
                    out=o[:], in0=p[:], scalar=1.0, in1=x_t[m][:, sl],
                    op0=ALU.mult, op1=ALU.add)
                nc.sync.dma_start(out=outT[m, :, sl], in_=o[:])
        stack.close()
    nc.compile()
    return nc


def prep_phase1_inputs(inputs, xs, core):
    b, k = divmod(core, 4)
    A = -np.exp(inputs['A_log'][k]).astype(np.float32)          # [512, 16]
    inb_full = (inputs['in_w'][k].astype(np.float32)
                @ inputs['norm_b'][k].astype(np.float32))       # [1024]
    return {
        "xT": np.ascontiguousarray(xs[b, k]).reshape(2, 128, L).astype(np.float32),
        "nwb": np.stack([inputs['norm_w'][k].reshape(2, 128),
                         inputs['norm_b'][k].reshape(2, 128)], 2).astype(np.float32),
        "inwT": _bf16(inputs['in_w'][k].T.reshape(2, 128, 2 * D_INNER)),
        "convw": inputs['conv_w'][k][:, 0, :].reshape(4, 128, D_CONV).astype(np.float32),
        "convb": inputs['conv_b'][k].reshape(4, 128, 1).astype(np.float32),
        "xprojT": _bf16(inputs['xproj_w'][k].T.reshape(4, 128, 48)),
        "dtwT": _bf16(inputs['dt_w'][k].T),
        "dtb": inputs['dt_b'][k].reshape(4, 128, 1).astype(np.float32),
        "AT": A.reshape(4, 128, D_STATE),
        "Dpt": inputs['Dp'][k].reshape(4, 128, 1).astype(np.float32),
        "outwT": _bf16(inputs['out_w'][k].T.reshape(4, 128, C)),
        "identT": _bf16(np.eye(128)),
        "inb": inb_full.reshape(8, 128).T.astype(np.float32).copy(),
        "oneM": _bf16(np.full((128, 128), 1.0 / 256.0)),
    }


def run_phase1_bass(inputs, xs):
    if "p1" not in _cache:
        _cache["p1"] = build_phase1()
    nc = _cache["p1"]
    ins = [prep_phase1_inputs(inputs, xs, core) for core in range(8)]
    res = run_bass_kernel_spmd(nc, ins, list(range(8)))
    return [res.results[c]["outT"].reshape(C, L) for c in range(8)]


# ---------------------------------------------------------------------------
# numpy reference fallback (kept for testing)
# ---------------------------------------------------------------------------

def _sigmoid(v):
    return 1.0 / (1.0 + np.exp(-v))


def mamba_block_np(xT, nw, nb, in_w, conv_w, conv_b, xproj_w, dt_w, dt_b,
                   A_log, Dp, out_w):
    x = xT.T
    mu = x.mean(-1, keepdims=True)
    var = ((x - mu) ** 2).mean(-1, keepdims=True)
    h = (x - mu) / np.sqrt(var + EPS) * nw + nb
    xz = h @ in_w.T
    xa, z = xz[:, :D_INNER], xz[:, D_INNER:]
    xa_t = xa.T
    w = conv_w[:, 0, :]
    pad = np.pad(xa_t, ((0, 0), (D_CONV - 1, 0)))
    conv = sum(pad[:, i:i + L] * w[:, i:i + 1] for i in range(D_CONV))
    u_t = conv + conv_b[:, None]
    u_t = u_t * _sigmoid(u_t)
    u = u_t.T
    dbc = u @ xproj_w.T
    dt_lr = dbc[:, :DT_RANK]
    Bm = dbc[:, DT_RANK:DT_RANK + D_STATE]
    Cm = dbc[:, DT_RANK + D_STATE:]
    vv = dt_lr @ dt_w.T + dt_b
    dt = np.log1p(np.exp(-np.abs(vv))) + np.maximum(vv, 0)
    A = -np.exp(A_log)
    dA = np.exp(dt[:, :, None] * A[None])
    dBu = dt[:, :, None] * Bm[:, None, :] * u[:, :, None]
    hs = np.zeros((D_INNER, D_STATE), np.float32)
    ys = np.zeros((L, D_INNER), np.float32)
    for t in range(L):
        hs = dA[t] * hs + dBu[t]
        ys[t] = (hs * Cm[t][None, :]).sum(-1)
    y = ys + Dp * u
    y = y * (z * _sigmoid(z))
    mo = y @ out_w.T
    return xT + mo.T


def run_phase1_np(inputs, xs):
    outs = []
    for core in range(8):
        b, k = divmod(core, 4)
        outs.append(mamba_block_np(
            np.ascontiguousarray(xs[b, k]), inputs['norm_w'][k], inputs['norm_b'][k],
            inputs['in_w'][k], inputs['conv_w'][k], inputs['conv_b'][k],
            inputs['xproj_w'][k], inputs['dt_w'][k], inputs['dt_b'][k],
            inputs['A_log'][k], inputs['Dp'][k], inputs['out_w'][k]))
    return outs


# ---------------------------------------------------------------------------
# top level
# ---------------------------------------------------------------------------

def kernel(**inputs):
    inputs = {k: np.asarray(v, np.float32) if np.asarray(v).dtype == np.float32
              else np.asarray(v) for k, v in inputs.items()}
    xs = scan_jego_np(inputs['feat0'], inputs['feat1'])  # [B,4,C,L]
    p1 = run_phase1(inputs, xs)
    ys = np.stack([np.stack(p1[4 * b:4 * b + 4], 0) for b in range(B)], 0)
    d0, d1 = merge_jego_np(ys, H8, W8)
    Dfull = np.concatenate([d0, d1], 0)
    desc = run_phase2(Dfull, inputs['glu_w'], inputs['glu_b'])
    dd0, dd1 = desc[:B], desc[B:]
    return np.stack([dd0.reshape(B, C, -1), dd1.reshape(B, C, -1)], 0).astype(np.float32)


def run_phase1(inputs, xs):
    return run_phase1_bass(inputs, xs)



# revision 44
# speedup vs baseline: 11662.0587x; 1.0644x over previous
"""Trainium2 Bass kernel for nn_JointMamba: 4-direction Mamba scan + GLU conv.

Sharding: phase 1 runs the 8 independent (batch, direction) Mamba blocks one
per NeuronCore; phase 2 reshards the merged feature maps over (image, row-half)
and runs the 3x3 GLU conv, one shard per core. Host does only permutations /
layout prep (scan_jego / merge_jego are pure index shuffles).
"""
import sys
import numpy as np

try:
    import concourse.bass as bass  # noqa: F401
except ImportError:
    sys.path.insert(0, "/opt/trn_rl_repo")

import concourse.bass as bass
import concourse.bacc as bacc
import concourse.mybir as mybir
from concourse.bass_utils import run_bass_kernel_spmd
from concourse import tile

F32 = mybir.dt.float32
BF16 = mybir.dt.bfloat16
ALU = mybir.AluOpType
AF = mybir.ActivationFunctionType

B, C, H8, W8 = 2, 256, 64, 64
D_INNER, D_STATE, D_CONV, DT_RANK = 512, 16, 4, 16
L = (H8 // 2) * W8  # 2048
EPS = 1e-5

_cache = {}


def _bf16(x):
    import ml_dtypes
    return np.asarray(x, dtype=ml_dtypes.bfloat16)


# ---------------------------------------------------------------------------
# host-side permutations (pure data movement)
# ---------------------------------------------------------------------------

def scan_jego_np(d0, d1):
    d2w = np.concatenate([d0, d1], 3)
    d2h = np.concatenate([d0, d1], 2)
    b, c = d0.shape[:2]
    x0 = d2w[:, :, ::2, ::2].reshape(b, c, -1)
    x1 = np.swapaxes(d2h, 2, 3)[:, :, 1::2, 1::2].reshape(b, c, -1)
    x2 = d2w[:, :, ::2, 1::2].reshape(b, c, -1)[:, :, ::-1]
    x3 = np.swapaxes(d2h, 2, 3)[:, :, ::2, 1::2].reshape(b, c, -1)[:, :, ::-1]
    return np.stack([x0, x1, x2, x3], 1)  # [B,4,C,L]


def merge_jego_np(ys, ori_h, ori_w):
    b, k, c, Lx = ys.shape
    H, W = ori_h // 2, ori_w // 2
    y2w = np.zeros((b, c, ori_h, 2 * ori_w), ys.dtype)
    y2h = np.zeros((b, c, 2 * ori_h, ori_w), ys.dtype)
    y2w[:, :, ::2, ::2] = ys[:, 0].reshape(b, c, H, 2 * W)
    y2h[:, :, 1::2, 1::2] = np.swapaxes(ys[:, 1].reshape(b, c, W, 2 * H), 2, 3)
    y2w[:, :, ::2, 1::2] = ys[:, 2][:, :, ::-1].reshape(b, c, H, 2 * W)
    y2h[:, :, 1::2, ::2] = np.swapaxes(ys[:, 3][:, :, ::-1].reshape(b, c, W, 2 * H), 2, 3)
    d0w, d1w = np.split(y2w, 2, axis=3)
    d0h, d1h = np.split(y2h, 2, axis=2)
    return d0w + d0h, d1w + d1h


# ---------------------------------------------------------------------------
# phase 2: 3x3 conv + GLU, sharded over (image, row-half)
# ---------------------------------------------------------------------------

def build_phase2():
    """Per core: dpad [2,128,34*66] bf16, wc [9,2,128,512] bf16 (lhsT per tap),
    bias [128,4] f32.  Output o [2,128,2048] f32 (= [256, 32, 64] GLU'd rows)."""
    nc = bacc.Bacc("TRN2", target_bir_lowering=False, debug=False, num_devices=8)
    dpad = nc.dram_tensor("dpad", [2, 128, 34 * 66], BF16, kind="ExternalInput")
    wc = nc.dram_tensor("wc", [9, 2, 128, 512], BF16, kind="ExternalInput")
    bias = nc.dram_tensor("bias", [128, 4], F32, kind="ExternalInput")
    out = nc.dram_tensor("o", [2, 128, 2048], F32, kind="ExternalOutput")

    with tile.TileContext(nc) as tc:
        with tc.tile_pool(name="cw", bufs=1) as cw, \
             tc.tile_pool(name="cd", bufs=1) as cd, \
             tc.tile_pool(name="cpsum", bufs=2, space="PSUM") as cpsum, \
             tc.tile_pool(name="cact", bufs=3) as cact:
            dbf = []
            for kc in range(2):
                d = cd.tile([128, 34 * 66], BF16, name=f"d{kc}")
                nc.sync.dma_start(out=d[:], in_=dpad[kc])
                dbf.append(d)
            wt = []
            for tap in range(9):
                row_w = []
                for kc in range(2):
                    w_ = cw.tile([128, 512], BF16, name=f"w{tap}_{kc}")
                    nc.sync.dma_start(out=w_[:], in_=wc[tap, kc])
                    row_w.append(w_)
                wt.append(row_w)
            bias_t = cw.tile([128, 4], F32, name="bias_t")
            nc.sync.dma_start(out=bias_t[:], in_=bias[:])

            for rg in range(4):  # row groups of 8 output rows
                ps = []
                for m in range(4):  # co tiles of 128
                    p = cpsum.tile([128, 512], F32, name=f"ps{m}")
                    ps.append(p)
                    for ti, tap in enumerate(range(9)):
                        dy, dx = divmod(tap, 3)
                        for kc in range(2):
                            rhs_bf = dbf[kc][:, (rg * 8 + dy) * 66 + dx:]
                            rhs_bf = bass.AP(rhs_bf.tensor, rhs_bf.offset,
                                             [rhs_bf.ap[0], [66, 8], [1, 64]])
                            nc.tensor.matmul(
                                p[:], lhsT=wt[tap][kc][:, m * 128:(m + 1) * 128],
                                rhs=rhs_bf, start=(ti == 0 and kc == 0),
                                stop=(tap == 8 and kc == 1))
                # GLU: a = ps[0..1], g = ps[2..3]
                for m in range(2):
                    sg = cact.tile([128, 512], F32, name="sg")
                    nc.scalar.activation(out=sg[:], in_=ps[2 + m][:],
                                         func=AF.Sigmoid, bias=bias_t[:, 2 + m:3 + m])
                    av = cact.tile([128, 512], F32, name="av")
                    nc.scalar.activation(out=av[:], in_=ps[m][:],
                                         func=AF.Identity, bias=bias_t[:, m:m + 1])
                    og = cact.tile([128, 512], F32, name="og")
                    nc.vector.tensor_tensor(out=og[:], in0=av[:], in1=sg[:], op=ALU.mult)
                    nc.sync.dma_start(out=out[m, :, rg * 512:(rg + 1) * 512], in_=og[:])
    with _restrict_act_tables({"sigmoid_and_others"}):
        nc.compile()
    return nc


def prep_phase2_weights(glu_w, glu_b):
    # wc[tap, kc, ci, co] = glu_w[co, kc*128+ci, dy, dx]
    w = np.transpose(glu_w, (2, 3, 1, 0)).reshape(9, 2, 128, 512)
    bias = glu_b.reshape(4, 128).T.copy()  # [128, 4] per-partition
    return _bf16(w), bias


def run_phase2(Dfull, glu_w, glu_b):
    """Dfull [4, 256, 64, 64] -> [4, 256, 64, 64] after conv+GLU."""
    if "p2" not in _cache:
        _cache["p2"] = build_phase2()
    nc = _cache["p2"]
    w_hi, bias = prep_phase2_weights(glu_w, glu_b)
    Dpad = np.pad(Dfull, ((0, 0), (0, 0), (1, 1), (1, 1)))
    ins = []
    for core in range(8):
        img, half = divmod(core, 2)
        r0 = half * 32
        dslice = Dpad[img, :, r0:r0 + 34, :].reshape(2, 128, 34 * 66)
        ins.append({"dpad": _bf16(dslice), "wc": w_hi, "bias": bias})
    res = run_bass_kernel_spmd(nc, ins, list(range(8)))
    out = np.zeros((4, 256, 64, 64), np.float32)
    for core in range(8):
        img, half = divmod(core, 2)
        o = res.results[core]["o"].reshape(256, 32, 64)
        out[img, :, half * 32:half * 32 + 32, :] = o
    return out


# ---------------------------------------------------------------------------
# phase 1: per-(b,k) Mamba block on one core
# layout: feature-major ([channel, t]) throughout; selective scan uses the
# native DVE TensorTensorScan along the free (t) axis, 16 state rows per
# d-block handled as independent [128, T] sweeps.
# ---------------------------------------------------------------------------

HC_GP8 = 0    # of every 8 hC multiplies, this many go to gpsimd (contention: keep 0)
DBU_GP8 = 0   # of every 8 dBu multiplies, this many go to gpsimd
HLAST_GP = False

import contextlib


@contextlib.contextmanager
def _restrict_act_tables(keep):
    """Force bacc's ACT-table chooser onto `keep` sets only (names/ids keep
    their positions so walrus still loads the right binaries).  Avoids
    per-instruction table thrash between e.g. exp_and_others / natural_log
    when natural_log_exp_and_others covers both."""
    import concourse.hw_specs as hw_specs
    orig = bacc.get_activation_tables

    def patched(arch):
        full = hw_specs.get_activation_tables(arch)
        return {name: (funcs if name in keep else set())
                for name, funcs in full.items()}

    bacc.get_activation_tables = patched
    try:
        yield
    finally:
        bacc.get_activation_tables = orig


def build_phase1():
    nc = bacc.Bacc("TRN2", target_bir_lowering=False, debug=False, num_devices=8)
    xT = nc.dram_tensor("xT", [2, 128, L], F32, kind="ExternalInput")
    nwb = nc.dram_tensor("nwb", [2, 128, 2], F32, kind="ExternalInput")      # nw, nb
    inwT = nc.dram_tensor("inwT", [2, 128, 2 * D_INNER], BF16, kind="ExternalInput")
    convw = nc.dram_tensor("convw", [4, 128, D_CONV], F32, kind="ExternalInput")
    convb = nc.dram_tensor("convb", [4, 128, 1], F32, kind="ExternalInput")
    xprojT = nc.dram_tensor("xprojT", [4, 128, 48], BF16, kind="ExternalInput")
    dtwT = nc.dram_tensor("dtwT", [16, D_INNER], BF16, kind="ExternalInput")
    dtb = nc.dram_tensor("dtb", [4, 128, 1], F32, kind="ExternalInput")
    AT = nc.dram_tensor("AT", [4, 128, D_STATE], F32, kind="ExternalInput")
    Dpt = nc.dram_tensor("Dpt", [4, 128, 1], F32, kind="ExternalInput")
    outwT = nc.dram_tensor("outwT", [4, 128, C], BF16, kind="ExternalInput")
    inb = nc.dram_tensor("inb", [128, 8], F32, kind="ExternalInput")  # in_w @ nb
    oneM = nc.dram_tensor("oneM", [128, 128], BF16, kind="ExternalInput")    # 1/256
    BmD = nc.dram_tensor("BmD", [16, L], BF16, kind="Internal")
    CmD = nc.dram_tensor("CmD", [16, L], BF16, kind="Internal")
    outT = nc.dram_tensor("outT", [2, 128, L], F32, kind="ExternalOutput")

    TT = 512  # t-tile for the scan stage (PSUM-bank bound)
    with tile.TileContext(nc) as tc:
        import contextlib
        stack = contextlib.ExitStack()
        wpool = stack.enter_context(tc.tile_pool(name="wpool", bufs=1))
        big = stack.enter_context(tc.tile_pool(name="big", bufs=1))
        ps = stack.enter_context(tc.tile_pool(name="ps", bufs=4, space="PSUM"))
        scr = stack.enter_context(tc.tile_pool(name="scr", bufs=1))
        bc = stack.enter_context(tc.tile_pool(name="bc", bufs=1))
        sc = stack.enter_context(tc.tile_pool(name="sc", bufs=3))
        cvp = stack.enter_context(tc.tile_pool(name="cvp", bufs=2))

        # ---- load inputs (x in per-chunk pieces so LN can start early)
        x_t = [wpool.tile([128, L], F32, name=f"x{i}") for i in range(2)]
        for nch in range(4):
            for i in range(2):
                nc.sync.dma_start(out=x_t[i][:, nch * 512:(nch + 1) * 512],
                                  in_=xT[i, :, nch * 512:(nch + 1) * 512])
        nwb_t = wpool.tile([128, 4], F32, name="nwb_t")
        for i in range(2):
            nc.sync.dma_start(out=nwb_t[:, 2 * i:2 * i + 2], in_=nwb[i])
        inw_t = [wpool.tile([128, 2 * D_INNER], BF16, name=f"inw{i}") for i in range(2)]
        for i in range(2):
            nc.sync.dma_start(out=inw_t[i][:], in_=inwT[i])
        convw_t = [wpool.tile([128, D_CONV], F32, name=f"cw{i}") for i in range(4)]
        convb_t = [wpool.tile([128, 1], F32, name=f"cb{i}") for i in range(4)]
        xproj_t = [wpool.tile([128, 48], BF16, name=f"xp{i}") for i in range(4)]
        dtb_t = [wpool.tile([128, 1], F32, name=f"dtb{i}") for i in range(4)]
        A_t = [wpool.tile([128, D_STATE], F32, name=f"A{i}") for i in range(4)]
        Dp_t = [wpool.tile([128, 1], F32, name=f"Dp{i}") for i in range(4)]
        outw_t = [wpool.tile([128, C], BF16, name=f"ow{i}") for i in range(4)]
        for i in range(4):
            nc.sync.dma_start(out=convw_t[i][:], in_=convw[i])
            nc.sync.dma_start(out=convb_t[i][:], in_=convb[i])
            nc.sync.dma_start(out=xproj_t[i][:], in_=xprojT[i])
            nc.sync.dma_start(out=dtb_t[i][:], in_=dtb[i])
            nc.sync.dma_start(out=A_t[i][:], in_=AT[i])
            nc.sync.dma_start(out=Dp_t[i][:], in_=Dpt[i])
            nc.sync.dma_start(out=outw_t[i][:], in_=outwT[i])
        dtw_t = wpool.tile([16, D_INNER], BF16, name="dtw_t")
        nc.sync.dma_start(out=dtw_t[:], in_=dtwT[:])
        eps_t = wpool.tile([128, 1], F32, name="eps_t")
        nc.vector.memset(eps_t[:], EPS)
        inb_t = wpool.tile([128, 8], F32, name="inb_t")
        nc.sync.dma_start(out=inb_t[:], in_=inb[:])
        oneM_t = wpool.tile([128, 128], BF16, name="oneM_t")
        nc.sync.dma_start(out=oneM_t[:], in_=oneM[:])

        # ---- layernorm fused with in-proj, chunked over t.
        # nb is folded into the in-proj bias (inb) on the host; x_ln = (x-mu)*inv*nw.
        xa_pad = [big.tile([128, 3 + L], BF16, name=f"xap{i}") for i in range(4)]
        for i in range(4):
            nc.vector.memset(xa_pad[i][:, 0:3], 0.0)
        sz = [big.tile([128, L], BF16, name=f"sz{i}") for i in range(4)]
        u_t = [big.tile([128, L], BF16, name=f"u{i}") for i in range(4)]
        for nch in range(4):
            sl = slice(nch * 512, (nch + 1) * 512)
            x_bf = [scr.tile([128, 512], BF16, name=f"xbf{i}", tag=f"xbf{i}")
                    for i in range(2)]
            sq_bf = [scr.tile([128, 512], BF16, name=f"sq{i}", tag=f"sq{i}")
                     for i in range(2)]
            for i in range(2):
                nc.scalar.activation(out=x_bf[i][:], in_=x_t[i][:, sl], func=AF.Copy)
                nc.scalar.activation(out=sq_bf[i][:], in_=x_t[i][:, sl], func=AF.Square)
            mu_p = ps.tile([128, 512], F32, name="mu_p", tag="mm")
            for i in range(2):
                nc.tensor.matmul(mu_p[:], lhsT=oneM_t[:], rhs=x_bf[i][:],
                                 start=(i == 0), stop=(i == 1))
            ss_p = ps.tile([128, 512], F32, name="ss_p", tag="mm")
            for i in range(2):
                nc.tensor.matmul(ss_p[:], lhsT=oneM_t[:], rhs=sq_bf[i][:],
                                 start=(i == 0), stop=(i == 1))
            # var = E[x^2] - mu^2 >= 0 ; inv = exp(-0.5*ln(var+eps))
            mu2 = scr.tile([128, 512], F32, name="mu2", tag="mu2")
            nc.scalar.activation(out=mu2[:], in_=mu_p[:], func=AF.Square)
            var_s = scr.tile([128, 512], F32, name="var_s", tag="var_s")
            nc.vector.scalar_tensor_tensor(
                out=var_s[:], in0=mu2[:], scalar=-1.0, in1=ss_p[:],
                op0=ALU.mult, op1=ALU.add)
            lnv = scr.tile([128, 512], F32, name="lnv", tag="lnv")
            nc.scalar.activation(out=lnv[:], in_=var_s[:], func=AF.Ln, bias=eps_t[:])
            inv_b = scr.tile([128, 512], BF16, name="inv_b", tag="inv_b")
            nc.scalar.activation(out=inv_b[:], in_=lnv[:], func=AF.Exp, scale=-0.5)
            x_ln = [scr.tile([128, 512], BF16, name=f"xln{i}", tag=f"xln{i}")
                    for i in range(2)]
            for i in range(2):
                cen = scr.tile([128, 512], BF16, name="cen", tag="cen")
                nc.vector.scalar_tensor_tensor(
                    out=cen[:], in0=mu_p[:], scalar=-1.0, in1=x_t[i][:, sl],
                    op0=ALU.mult, op1=ALU.add)
                nc.vector.scalar_tensor_tensor(
                    out=x_ln[i][:], in0=cen[:], scalar=nwb_t[:, 2 * i:2 * i + 1],
                    in1=inv_b[:], op0=ALU.mult, op1=ALU.mult)
            for m in range(8):
                p = ps.tile([128, 512], F32, name="inp_p", tag="mm")
                for i in range(2):
                    nc.tensor.matmul(p[:], lhsT=inw_t[i][:, m * 128:(m + 1) * 128],
                                     rhs=x_ln[i][:], start=(i == 0), stop=(i == 1))
                if m < 4:
                    nc.scalar.activation(out=xa_pad[m][:, 3 + nch * 512:3 + (nch + 1) * 512],
                                         in_=p[:], func=AF.Identity, bias=inb_t[:, m:m + 1])
                else:
                    # raw z for now; silu applied late (different ACT table set)
                    nc.scalar.activation(out=sz[m - 4][:, sl], in_=p[:],
                                         func=AF.Identity, bias=inb_t[:, m:m + 1])
            # depthwise causal conv(4) chunk (silu later): tensor_scalar
            # tap-multiplies (4x mode) + tensor_tensor adds (2x).
            base = nch * 512  # xa_pad column base (pad offset 3 built in)
            for i in range(4):
                t0 = cvp.tile([128, 512], BF16, name="t0", tag="convt0")
                nc.vector.tensor_scalar_mul(out=t0[:], in0=xa_pad[i][:, base:base + 512],
                                            scalar1=convw_t[i][:, 0:1])
                t1 = cvp.tile([128, 512], BF16, name="t1", tag="convt1")
                nc.vector.tensor_scalar_mul(out=t1[:], in0=xa_pad[i][:, base + 1:base + 513],
                                            scalar1=convw_t[i][:, 1:2])
                nc.vector.tensor_tensor(out=t0[:], in0=t0[:], in1=t1[:], op=ALU.add)
                nc.vector.tensor_scalar_mul(out=t1[:], in0=xa_pad[i][:, base + 2:base + 514],
                                            scalar1=convw_t[i][:, 2:3])
                t2 = cvp.tile([128, 512], BF16, name="t2", tag="convt2")
                nc.vector.tensor_scalar_mul(out=t2[:], in0=xa_pad[i][:, base + 3:base + 515],
                                            scalar1=convw_t[i][:, 3:4])
                nc.vector.tensor_tensor(out=t1[:], in0=t1[:], in1=t2[:], op=ALU.add)
                nc.vector.tensor_tensor(out=u_t[i][:, nch * 512:(nch + 1) * 512],
                                        in0=t0[:], in1=t1[:], op=ALU.add)

        # u = silu(u + convb): on the critical path into xproj, batched so the
        # scheduler pays at most one table round-trip.
        for i in range(4):
            nc.scalar.activation(out=u_t[i][:], in_=u_t[i][:], func=AF.Silu,
                                 bias=convb_t[i][:])

        # ---- xproj -> dt_lr (partitions 0:16), B (16:32), C (32:48)
        dbc_all = big.tile([48, L], BF16, name="dbc_all")
        for nch in range(4):
            sl = slice(nch * 512, (nch + 1) * 512)
            dbc_p = ps.tile([48, 512], F32, name="dbc_p", tag="mm")
            for i in range(4):
                nc.tensor.matmul(dbc_p[:], lhsT=xproj_t[i][:], rhs=u_t[i][:, sl],
                                 start=(i == 0), stop=(i == 3))
            nc.scalar.activation(out=dbc_all[:, sl], in_=dbc_p[:], func=AF.Copy)
            # bounce B/C rows through DRAM so they can be partition-broadcast
            # back by pure DMA (stride-0 partition APs) during the scan.
            nc.sync.dma_start(out=BmD[:, sl], in_=dbc_all[16:32, sl])
            nc.sync.dma_start(out=CmD[:, sl], in_=dbc_all[32:48, sl])

        # ---- dt = softplus(dt_w @ dt_lr + dt_b)  [512, L] bf16
        dt_t = [big.tile([128, L], BF16, name=f"dt{i}") for i in range(4)]
        dtu_t = [big.tile([128, L], BF16, name=f"dtu{i}") for i in range(4)]
        for m in range(4):
            for nch in range(4):
                sl = slice(nch * 512, (nch + 1) * 512)
                p = ps.tile([128, 512], F32, name="dt_p", tag="mm")
                nc.tensor.matmul(p[:], lhsT=dtw_t[:, m * 128:(m + 1) * 128],
                                 rhs=dbc_all[0:16, sl], start=True, stop=True)
                # softplus(v) = ln(1 + exp(v)) (no softplus in the ACT tables;
                # v stays small here so exp cannot overflow)
                e_s = scr.tile([128, 512], F32, name="e_s", tag="e_s")
                nc.scalar.activation(out=e_s[:], in_=p[:], func=AF.Exp,
                                     bias=dtb_t[m][:])
                nc.scalar.activation(out=dt_t[m][:, sl], in_=e_s[:], func=AF.Ln,
                                     bias=1.0)
        for m in range(4):
            nc.vector.tensor_tensor(out=dtu_t[m][:], in0=dt_t[m][:], in1=u_t[m][:],
                                    op=ALU.mult)

        # ---- selective scan, full-L n-outer: per state n, DMA-broadcast the
        # B/C rows once, then per d-block do exp/mult/scan/mult/accumulate as
        # single [128, L] ops.  Scan scratch reuses the (dead) xa_pad slots.
        y_acc = [big.tile([128, L], BF16, name=f"ya{i}") for i in range(4)]
        for n in range(16):
            bb = bc.tile([128, L], BF16, name="bbF", tag=f"bbF{n % 2}")
            nc.sync.dma_start(out=bb[:], in_=bass.AP(BmD, n * L, [[0, 128], [1, L]]))
            cb = bc.tile([128, L], BF16, name="cbF", tag=f"cbF{n % 2}")
            nc.sync.dma_start(out=cb[:], in_=bass.AP(CmD, n * L, [[0, 128], [1, L]]))
            for dblk in range(4):
                par = (n * 4 + dblk) % 2
                dA = big.tile([128, L], BF16, name="dA_s", tag=f"da{(n * 4 + dblk) % 3}")
                nc.scalar.activation(out=dA[:], in_=dt_t[dblk][:], func=AF.Exp,
                                     scale=A_t[dblk][:, n:n + 1])
                dBu = big.tile([128, L], BF16, name="dBu_s", tag=f"xap{par}")
                nc.vector.tensor_tensor(out=dBu[:], in0=dtu_t[dblk][:], in1=bb[:],
                                        op=ALU.mult)
                h = big.tile([128, L], BF16, name="h_s", tag=f"xap{2 + par}")
                nc.vector.tensor_tensor_scan(
                    out=h[:], data0=dA[:], data1=dBu[:],
                    initial=0.0, op0=ALU.mult, op1=ALU.add)
                if n == 0:
                    nc.vector.tensor_tensor(out=y_acc[dblk][:], in0=h[:],
                                            in1=cb[:], op=ALU.mult)
                else:
                    hc = big.tile([128, L], BF16, name="hc_s", tag=f"hcs{par}")
                    nc.vector.tensor_tensor(out=hc[:], in0=h[:], in1=cb[:],
                                            op=ALU.mult)
                    nc.vector.tensor_tensor(out=y_acc[dblk][:], in0=y_acc[dblk][:],
                                            in1=hc[:], op=ALU.add)

        # sz = silu(z), forced late via a zero "trigger" bias that depends on
        # y_acc (keeps the ACT table set stable during the scan's exp storm).
        trig = wpool.tile([128, 1], F32, name="trig")
        nc.vector.tensor_scalar_mul(out=trig[:], in0=y_acc[0][:, 0:1], scalar1=0.0)
        for m in range(4):
            nc.scalar.activation(out=sz[m][:], in_=sz[m][:], func=AF.Silu,
                                 bias=trig[:])

        # ---- y = (y_acc + Dp*u) * silu(z) (in place); out = out_w @ y + x
        yg = y_acc
        for m in range(4):
            nc.vector.scalar_tensor_tensor(
                out=y_acc[m][:], in0=u_t[m][:], scalar=Dp_t[m][:], in1=y_acc[m][:],
                op0=ALU.mult, op1=ALU.add)
            nc.vector.tensor_tensor(out=y_acc[m][:], in0=y_acc[m][:], in1=sz[m][:],
                                    op=ALU.mult)
        for m in range(2):
            for nch in range(4):
                sl = slice(nch * 512, (nch + 1) * 512)
                p = ps.tile([128, 512], F32, name="out_p", tag="mm")
                for i in range(4):
                    nc.tensor.matmul(p[:], lhsT=outw_t[i][:, m * 128:(m + 1) * 128],
                                     rhs=yg[i][:, sl], start=(i == 0), stop=(i == 3))
                o = sc.tile([128, 512], F32, name="o", tag="outsc")
                nc.vector.scalar_tensor_tensor(
                    out=o[:], in0=p[:], scalar=1.0, in1=x_t[m][:, sl],
                    op0=ALU.mult, op1=ALU.add)
                nc.sync.dma_start(out=outT[m, :, sl], in_=o[:])
        stack.close()
    with _restrict_act_tables({"natural_log_exp_and_others", "silu_and_others"}):
        nc.compile()
    return nc


def prep_phase1_inputs(inputs, xs, core):
    b, k = divmod(core, 4)
    wkey = ("p1w", k, id(inputs['in_w']), id(inputs['A_log']))
    if wkey in _cache:
        d = dict(_cache[wkey])
        d["xT"] = np.ascontiguousarray(xs[b, k]).reshape(2, 128, L).astype(np.float32)
        return d
    A = -np.exp(inputs['A_log'][k]).astype(np.float32)          # [512, 16]
    inb_full = (inputs['in_w'][k].astype(np.float32)
                @ inputs['norm_b'][k].astype(np.float32))       # [1024]
    w = {
        "nwb": np.stack([inputs['norm_w'][k].reshape(2, 128),
                         inputs['norm_b'][k].reshape(2, 128)], 2).astype(np.float32),
        "inwT": _bf16(inputs['in_w'][k].T.reshape(2, 128, 2 * D_INNER)),
        "convw": inputs['conv_w'][k][:, 0, :].reshape(4, 128, D_CONV).astype(np.float32),
        "convb": inputs['conv_b'][k].reshape(4, 128, 1).astype(np.float32),
        "xprojT": _bf16(inputs['xproj_w'][k].T.reshape(4, 128, 48)),
        "dtwT": _bf16(inputs['dt_w'][k].T),
        "dtb": inputs['dt_b'][k].reshape(4, 128, 1).astype(np.float32),
        "AT": A.reshape(4, 128, D_STATE),
        "Dpt": inputs['Dp'][k].reshape(4, 128, 1).astype(np.float32),
        "outwT": _bf16(inputs['out_w'][k].T.reshape(4, 128, C)),
        "inb": inb_full.reshape(8, 128).T.astype(np.float32).copy(),
        "oneM": _bf16(np.full((128, 128), 1.0 / 256.0)),
    }
    _cache[wkey] = w
    d = dict(w)
    d["xT"] = np.ascontiguousarray(xs[b, k]).reshape(2, 128, L).astype(np.float32)
    return d


def run_phase1_bass(inputs, xs):
    if "p1" not in _cache:
        _cache["p1"] = build_phase1()
    nc = _cache["p1"]
    ins = [prep_phase1_inputs(inputs, xs, core) for core in range(8)]
    res = run_bass_kernel_spmd(nc, ins, list(range(8)))
    return [res.results[c]["outT"].reshape(C, L) for c in range(8)]


# ---------------------------------------------------------------------------
# numpy reference fallback (kept for testing)
# ---------------------------------------------------------------------------

def _sigmoid(v):
    return 1.0 / (1.0 + np.exp(-v))


def mamba_block_np(xT, nw, nb, in_w, conv_w, conv_b, xproj_w, dt_w, dt_b,
                   A_log, Dp, out_w):
    x = xT.T
    mu = x.mean(-1, keepdims=True)
    var = ((x - mu) ** 2).mean(-1, keepdims=True)
    h = (x - mu) / np.sqrt(var + EPS) * nw + nb
    xz = h @ in_w.T
    xa, z = xz[:, :D_INNER], xz[:, D_INNER:]
    xa_t = xa.T
    w = conv_w[:, 0, :]
    pad = np.pad(xa_t, ((0, 0), (D_CONV - 1, 0)))
    conv = sum(pad[:, i:i + L] * w[:, i:i + 1] for i in range(D_CONV))
    u_t = conv + conv_b[:, None]
    u_t = u_t * _sigmoid(u_t)
    u = u_t.T
    dbc = u @ xproj_w.T
    dt_lr = dbc[:, :DT_RANK]
    Bm = dbc[:, DT_RANK:DT_RANK + D_STATE]
    Cm = dbc[:, DT_RANK + D_STATE:]
    vv = dt_lr @ dt_w.T + dt_b
    dt = np.log1p(np.exp(-np.abs(vv))) + np.maximum(vv, 0)
    A = -np.exp(A_log)
    dA = np.exp(dt[:, :, None] * A[None])
    dBu = dt[:, :, None] * Bm[:, None, :] * u[:, :, None]
    hs = np.zeros((D_INNER, D_STATE), np.float32)
    ys = np.zeros((L, D_INNER), np.float32)
    for t in range(L):
        hs = dA[t] * hs + dBu[t]
        ys[t] = (hs * Cm[t][None, :]).sum(-1)
    y = ys + Dp * u
    y = y * (z * _sigmoid(z))
    mo = y @ out_w.T
    return xT + mo.T


def run_phase1_np(inputs, xs):
    outs = []
    for core in range(8):
        b, k = divmod(core, 4)
        outs.append(mamba_block_np(
            np.ascontiguousarray(xs[b, k]), inputs['norm_w'][k], inputs['norm_b'][k],
            inputs['in_w'][k], inputs['conv_w'][k], inputs['conv_b'][k],
            inputs['xproj_w'][k], inputs['dt_w'][k], inputs['dt_b'][k],
            inputs['A_log'][k], inputs['Dp'][k], inputs['out_w'][k]))
    return outs


# ---------------------------------------------------------------------------
# top level
# ---------------------------------------------------------------------------

def kernel(**inputs):
    inputs = {k: np.asarray(v, np.float32) if np.asarray(v).dtype == np.float32
              else np.asarray(v) for k, v in inputs.items()}
    xs = scan_jego_np(inputs['feat0'], inputs['feat1'])  # [B,4,C,L]
    p1 = run_phase1(inputs, xs)
    ys = np.stack([np.stack(p1[4 * b:4 * b + 4], 0) for b in range(B)], 0)
    d0, d1 = merge_jego_np(ys, H8, W8)
    Dfull = np.concatenate([d0, d1], 0)
    desc = run_phase2(Dfull, inputs['glu_w'], inputs['glu_b'])
    dd0, dd1 = desc[:B], desc[B:]
    return np.stack([dd0.reshape(B, C, -1), dd1.reshape(B, C, -1)], 0).astype(np.float32)


def run_phase1(inputs, xs):
    return run_phase1_bass(inputs, xs)



# revision 46
# speedup vs baseline: 12034.5668x; 1.0319x over previous
"""Trainium2 Bass kernel for nn_JointMamba: 4-direction Mamba scan + GLU conv.

Sharding: phase 1 runs the 8 independent (batch, direction) Mamba blocks one
per NeuronCore; phase 2 reshards the merged feature maps over (image, row-half)
and runs the 3x3 GLU conv, one shard per core. Host does only permutations /
layout prep (scan_jego / merge_jego are pure index shuffles).
"""
import sys
import numpy as np

try:
    import concourse.bass as bass  # noqa: F401
except ImportError:
    sys.path.insert(0, "/opt/trn_rl_repo")

import concourse.bass as bass
import concourse.bacc as bacc
import concourse.mybir as mybir
from concourse.bass_utils import run_bass_kernel_spmd
from concourse import tile

F32 = mybir.dt.float32
BF16 = mybir.dt.bfloat16
ALU = mybir.AluOpType
AF = mybir.ActivationFunctionType

B, C, H8, W8 = 2, 256, 64, 64
D_INNER, D_STATE, D_CONV, DT_RANK = 512, 16, 4, 16
L = (H8 // 2) * W8  # 2048
EPS = 1e-5

_cache = {}


def _bf16(x):
    import ml_dtypes
    return np.asarray(x, dtype=ml_dtypes.bfloat16)


# ---------------------------------------------------------------------------
# host-side permutations (pure data movement)
# ---------------------------------------------------------------------------

def scan_jego_np(d0, d1):
    d2w = np.concatenate([d0, d1], 3)
    d2h = np.concatenate([d0, d1], 2)
    b, c = d0.shape[:2]
    x0 = d2w[:, :, ::2, ::2].reshape(b, c, -1)
    x1 = np.swapaxes(d2h, 2, 3)[:, :, 1::2, 1::2].reshape(b, c, -1)
    x2 = d2w[:, :, ::2, 1::2].reshape(b, c, -1)[:, :, ::-1]
    x3 = np.swapaxes(d2h, 2, 3)[:, :, ::2, 1::2].reshape(b, c, -1)[:, :, ::-1]
    return np.stack([x0, x1, x2, x3], 1)  # [B,4,C,L]


def merge_jego_np(ys, ori_h, ori_w):
    b, k, c, Lx = ys.shape
    H, W = ori_h // 2, ori_w // 2
    y2w = np.zeros((b, c, ori_h, 2 * ori_w), ys.dtype)
    y2h = np.zeros((b, c, 2 * ori_h, ori_w), ys.dtype)
    y2w[:, :, ::2, ::2] = ys[:, 0].reshape(b, c, H, 2 * W)
    y2h[:, :, 1::2, 1::2] = np.swapaxes(ys[:, 1].reshape(b, c, W, 2 * H), 2, 3)
    y2w[:, :, ::2, 1::2] = ys[:, 2][:, :, ::-1].reshape(b, c, H, 2 * W)
    y2h[:, :, 1::2, ::2] = np.swapaxes(ys[:, 3][:, :, ::-1].reshape(b, c, W, 2 * H), 2, 3)
    d0w, d1w = np.split(y2w, 2, axis=3)
    d0h, d1h = np.split(y2h, 2, axis=2)
    return d0w + d0h, d1w + d1h


# ---------------------------------------------------------------------------
# phase 2: 3x3 conv + GLU, sharded over (image, row-half)
# ---------------------------------------------------------------------------

def build_phase2():
    """Per core: dpad [2,128,34*66] bf16, wc [9,2,128,512] bf16 (lhsT per tap),
    bias [128,4] f32.  Output o [2,128,2048] f32 (= [256, 32, 64] GLU'd rows)."""
    nc = bacc.Bacc("TRN2", target_bir_lowering=False, debug=False, num_devices=8)
    dpad = nc.dram_tensor("dpad", [2, 128, 34 * 66], BF16, kind="ExternalInput")
    wc = nc.dram_tensor("wc", [9, 2, 128, 512], BF16, kind="ExternalInput")
    bias = nc.dram_tensor("bias", [128, 4], F32, kind="ExternalInput")
    out = nc.dram_tensor("o", [2, 128, 2048], F32, kind="ExternalOutput")

    with tile.TileContext(nc) as tc:
        with tc.tile_pool(name="cw", bufs=1) as cw, \
             tc.tile_pool(name="cd", bufs=1) as cd, \
             tc.tile_pool(name="cpsum", bufs=2, space="PSUM") as cpsum, \
             tc.tile_pool(name="cact", bufs=3) as cact:
            dbf = []
            for kc in range(2):
                d = cd.tile([128, 34 * 66], BF16, name=f"d{kc}")
                nc.sync.dma_start(out=d[:], in_=dpad[kc])
                dbf.append(d)
            wt = []
            for tap in range(9):
                row_w = []
                for kc in range(2):
                    w_ = cw.tile([128, 512], BF16, name=f"w{tap}_{kc}")
                    nc.sync.dma_start(out=w_[:], in_=wc[tap, kc])
                    row_w.append(w_)
                wt.append(row_w)
            bias_t = cw.tile([128, 4], F32, name="bias_t")
            nc.sync.dma_start(out=bias_t[:], in_=bias[:])

            for rg in range(4):  # row groups of 8 output rows
                ps = []
                for m in range(4):  # co tiles of 128
                    p = cpsum.tile([128, 512], F32, name=f"ps{m}")
                    ps.append(p)
                    for ti, tap in enumerate(range(9)):
                        dy, dx = divmod(tap, 3)
                        for kc in range(2):
                            rhs_bf = dbf[kc][:, (rg * 8 + dy) * 66 + dx:]
                            rhs_bf = bass.AP(rhs_bf.tensor, rhs_bf.offset,
                                             [rhs_bf.ap[0], [66, 8], [1, 64]])
                            nc.tensor.matmul(
                                p[:], lhsT=wt[tap][kc][:, m * 128:(m + 1) * 128],
                                rhs=rhs_bf, start=(ti == 0 and kc == 0),
                                stop=(tap == 8 and kc == 1))
                # GLU: a = ps[0..1], g = ps[2..3]
                for m in range(2):
                    sg = cact.tile([128, 512], F32, name="sg")
                    nc.scalar.activation(out=sg[:], in_=ps[2 + m][:],
                                         func=AF.Sigmoid, bias=bias_t[:, 2 + m:3 + m])
                    av = cact.tile([128, 512], F32, name="av")
                    nc.scalar.activation(out=av[:], in_=ps[m][:],
                                         func=AF.Identity, bias=bias_t[:, m:m + 1])
                    og = cact.tile([128, 512], F32, name="og")
                    nc.vector.tensor_tensor(out=og[:], in0=av[:], in1=sg[:], op=ALU.mult)
                    nc.sync.dma_start(out=out[m, :, rg * 512:(rg + 1) * 512], in_=og[:])
    with _restrict_act_tables({"sigmoid_and_others"}):
        nc.compile()
    return nc


def prep_phase2_weights(glu_w, glu_b):
    # wc[tap, kc, ci, co] = glu_w[co, kc*128+ci, dy, dx]
    w = np.transpose(glu_w, (2, 3, 1, 0)).reshape(9, 2, 128, 512)
    bias = glu_b.reshape(4, 128).T.copy()  # [128, 4] per-partition
    return _bf16(w), bias


def run_phase2(Dfull, glu_w, glu_b):
    """Dfull [4, 256, 64, 64] -> [4, 256, 64, 64] after conv+GLU."""
    if "p2" not in _cache:
        _cache["p2"] = build_phase2()
    nc = _cache["p2"]
    w_hi, bias = prep_phase2_weights(glu_w, glu_b)
    Dpad = np.pad(Dfull, ((0, 0), (0, 0), (1, 1), (1, 1)))
    ins = []
    for core in range(8):
        img, half = divmod(core, 2)
        r0 = half * 32
        dslice = Dpad[img, :, r0:r0 + 34, :].reshape(2, 128, 34 * 66)
        ins.append({"dpad": _bf16(dslice), "wc": w_hi, "bias": bias})
    res = run_bass_kernel_spmd(nc, ins, list(range(8)))
    out = np.zeros((4, 256, 64, 64), np.float32)
    for core in range(8):
        img, half = divmod(core, 2)
        o = res.results[core]["o"].reshape(256, 32, 64)
        out[img, :, half * 32:half * 32 + 32, :] = o
    return out


# ---------------------------------------------------------------------------
# phase 1: per-(b,k) Mamba block on one core
# layout: feature-major ([channel, t]) throughout; selective scan uses the
# native DVE TensorTensorScan along the free (t) axis, 16 state rows per
# d-block handled as independent [128, T] sweeps.
# ---------------------------------------------------------------------------

HC_GP8 = 0    # of every 8 hC multiplies, this many go to gpsimd (contention: keep 0)
DBU_GP8 = 0   # of every 8 dBu multiplies, this many go to gpsimd
HLAST_GP = False

import contextlib


@contextlib.contextmanager
def _restrict_act_tables(keep):
    """Force bacc's ACT-table chooser onto `keep` sets only (names/ids keep
    their positions so walrus still loads the right binaries).  Avoids
    per-instruction table thrash between e.g. exp_and_others / natural_log
    when natural_log_exp_and_others covers both."""
    import concourse.hw_specs as hw_specs
    orig = bacc.get_activation_tables

    def patched(arch):
        full = hw_specs.get_activation_tables(arch)
        return {name: (funcs if name in keep else set())
                for name, funcs in full.items()}

    bacc.get_activation_tables = patched
    try:
        yield
    finally:
        bacc.get_activation_tables = orig


def build_phase1():
    nc = bacc.Bacc("TRN2", target_bir_lowering=False, debug=False, num_devices=8)
    xT = nc.dram_tensor("xT", [2, 128, L], F32, kind="ExternalInput")
    nwb = nc.dram_tensor("nwb", [2, 128, 2], F32, kind="ExternalInput")      # nw, nb
    inwT = nc.dram_tensor("inwT", [2, 128, 2 * D_INNER], BF16, kind="ExternalInput")
    convw = nc.dram_tensor("convw", [4, 128, D_CONV], F32, kind="ExternalInput")
    convb = nc.dram_tensor("convb", [4, 128, 1], F32, kind="ExternalInput")
    xprojT = nc.dram_tensor("xprojT", [4, 128, 48], BF16, kind="ExternalInput")
    dtwT = nc.dram_tensor("dtwT", [16, D_INNER], BF16, kind="ExternalInput")
    dtb = nc.dram_tensor("dtb", [4, 128, 1], F32, kind="ExternalInput")
    AT = nc.dram_tensor("AT", [4, 128, D_STATE], F32, kind="ExternalInput")
    Dpt = nc.dram_tensor("Dpt", [4, 128, 1], F32, kind="ExternalInput")
    outwT = nc.dram_tensor("outwT", [4, 128, C], BF16, kind="ExternalInput")
    inb = nc.dram_tensor("inb", [128, 8], F32, kind="ExternalInput")  # in_w @ nb
    oneM = nc.dram_tensor("oneM", [128, 128], BF16, kind="ExternalInput")    # 1/256
    BmD = nc.dram_tensor("BmD", [16, L], BF16, kind="Internal")
    CmD = nc.dram_tensor("CmD", [16, L], BF16, kind="Internal")
    outT = nc.dram_tensor("outT", [2, 128, L], F32, kind="ExternalOutput")

    TT = 512  # t-tile for the scan stage (PSUM-bank bound)
    with tile.TileContext(nc) as tc:
        import contextlib
        stack = contextlib.ExitStack()
        wpool = stack.enter_context(tc.tile_pool(name="wpool", bufs=1))
        big = stack.enter_context(tc.tile_pool(name="big", bufs=1))
        ps = stack.enter_context(tc.tile_pool(name="ps", bufs=4, space="PSUM"))
        scr = stack.enter_context(tc.tile_pool(name="scr", bufs=1))
        bc = stack.enter_context(tc.tile_pool(name="bc", bufs=1))
        sc = stack.enter_context(tc.tile_pool(name="sc", bufs=3))
        cvp = stack.enter_context(tc.tile_pool(name="cvp", bufs=2))

        # ---- load inputs; LN-critical constants first, then x in per-chunk
        # pieces so the LN stats pipeline can start ~5us in.
        oneM_t = wpool.tile([128, 128], BF16, name="oneM_t")
        nc.sync.dma_start(out=oneM_t[:], in_=oneM[:])
        nwb_t = wpool.tile([128, 4], F32, name="nwb_t")
        for i in range(2):
            nc.sync.dma_start(out=nwb_t[:, 2 * i:2 * i + 2], in_=nwb[i])
        inb_t = wpool.tile([128, 8], F32, name="inb_t")
        nc.sync.dma_start(out=inb_t[:], in_=inb[:])
        x_t = [wpool.tile([128, L], F32, name=f"x{i}") for i in range(2)]
        for nch in range(4):
            for i in range(2):
                nc.sync.dma_start(out=x_t[i][:, nch * 512:(nch + 1) * 512],
                                  in_=xT[i, :, nch * 512:(nch + 1) * 512])
        inw_t = [wpool.tile([128, 2 * D_INNER], BF16, name=f"inw{i}") for i in range(2)]
        for i in range(2):
            nc.sync.dma_start(out=inw_t[i][:], in_=inwT[i])
        convw_t = [wpool.tile([128, D_CONV], F32, name=f"cw{i}") for i in range(4)]
        convb_t = [wpool.tile([128, 1], F32, name=f"cb{i}") for i in range(4)]
        xproj_t = [wpool.tile([128, 48], BF16, name=f"xp{i}") for i in range(4)]
        dtb_t = [wpool.tile([128, 1], F32, name=f"dtb{i}") for i in range(4)]
        A_t = [wpool.tile([128, D_STATE], F32, name=f"A{i}") for i in range(4)]
        Dp_t = [wpool.tile([128, 1], F32, name=f"Dp{i}") for i in range(4)]
        outw_t = [wpool.tile([128, C], BF16, name=f"ow{i}") for i in range(4)]
        for i in range(4):
            nc.sync.dma_start(out=convw_t[i][:], in_=convw[i])
            nc.sync.dma_start(out=convb_t[i][:], in_=convb[i])
            nc.sync.dma_start(out=xproj_t[i][:], in_=xprojT[i])
            nc.sync.dma_start(out=dtb_t[i][:], in_=dtb[i])
            nc.sync.dma_start(out=A_t[i][:], in_=AT[i])
            nc.sync.dma_start(out=Dp_t[i][:], in_=Dpt[i])
            nc.sync.dma_start(out=outw_t[i][:], in_=outwT[i])
        dtw_t = wpool.tile([16, D_INNER], BF16, name="dtw_t")
        nc.sync.dma_start(out=dtw_t[:], in_=dtwT[:])
        eps_t = wpool.tile([128, 1], F32, name="eps_t")
        nc.vector.memset(eps_t[:], EPS)

        # ---- layernorm fused with in-proj, chunked over t.
        # nb is folded into the in-proj bias (inb) on the host; x_ln = (x-mu)*inv*nw.
        xa_pad = [big.tile([128, 3 + L], BF16, name=f"xap{i}") for i in range(4)]
        for i in range(4):
            nc.vector.memset(xa_pad[i][:, 0:3], 0.0)
        sz = [big.tile([128, L], BF16, name=f"sz{i}") for i in range(4)]
        u_t = [big.tile([128, L], BF16, name=f"u{i}") for i in range(4)]
        for nch in range(4):
            sl = slice(nch * 512, (nch + 1) * 512)
            x_bf = [scr.tile([128, 512], BF16, name=f"xbf{i}", tag=f"xbf{i}")
                    for i in range(2)]
            sq_bf = [scr.tile([128, 512], BF16, name=f"sq{i}", tag=f"sq{i}")
                     for i in range(2)]
            for i in range(2):
                nc.scalar.activation(out=x_bf[i][:], in_=x_t[i][:, sl], func=AF.Copy)
                nc.scalar.activation(out=sq_bf[i][:], in_=x_t[i][:, sl], func=AF.Square)
            mu_p = ps.tile([128, 512], F32, name="mu_p", tag="mm")
            for i in range(2):
                nc.tensor.matmul(mu_p[:], lhsT=oneM_t[:], rhs=x_bf[i][:],
                                 start=(i == 0), stop=(i == 1))
            ss_p = ps.tile([128, 512], F32, name="ss_p", tag="mm")
            for i in range(2):
                nc.tensor.matmul(ss_p[:], lhsT=oneM_t[:], rhs=sq_bf[i][:],
                                 start=(i == 0), stop=(i == 1))
            # var = E[x^2] - mu^2 >= 0 ; inv = exp(-0.5*ln(var+eps))
            mu2 = scr.tile([128, 512], F32, name="mu2", tag="mu2")
            nc.scalar.activation(out=mu2[:], in_=mu_p[:], func=AF.Square)
            var_s = scr.tile([128, 512], F32, name="var_s", tag="var_s")
            nc.vector.scalar_tensor_tensor(
                out=var_s[:], in0=mu2[:], scalar=-1.0, in1=ss_p[:],
                op0=ALU.mult, op1=ALU.add)
            lnv = scr.tile([128, 512], F32, name="lnv", tag="lnv")
            nc.scalar.activation(out=lnv[:], in_=var_s[:], func=AF.Ln, bias=eps_t[:])
            inv_b = scr.tile([128, 512], BF16, name="inv_b", tag="inv_b")
            nc.scalar.activation(out=inv_b[:], in_=lnv[:], func=AF.Exp, scale=-0.5)
            x_ln = [scr.tile([128, 512], BF16, name=f"xln{i}", tag=f"xln{i}")
                    for i in range(2)]
            for i in range(2):
                cen = scr.tile([128, 512], BF16, name="cen", tag="cen")
                nc.vector.scalar_tensor_tensor(
                    out=cen[:], in0=mu_p[:], scalar=-1.0, in1=x_t[i][:, sl],
                    op0=ALU.mult, op1=ALU.add)
                nc.vector.scalar_tensor_tensor(
                    out=x_ln[i][:], in0=cen[:], scalar=nwb_t[:, 2 * i:2 * i + 1],
                    in1=inv_b[:], op0=ALU.mult, op1=ALU.mult)
            for m in range(8):
                p = ps.tile([128, 512], F32, name="inp_p", tag="mm")
                for i in range(2):
                    nc.tensor.matmul(p[:], lhsT=inw_t[i][:, m * 128:(m + 1) * 128],
                                     rhs=x_ln[i][:], start=(i == 0), stop=(i == 1))
                if m < 4:
                    nc.scalar.activation(out=xa_pad[m][:, 3 + nch * 512:3 + (nch + 1) * 512],
                                         in_=p[:], func=AF.Identity, bias=inb_t[:, m:m + 1])
                else:
                    # raw z for now; silu applied late (different ACT table set)
                    nc.scalar.activation(out=sz[m - 4][:, sl], in_=p[:],
                                         func=AF.Identity, bias=inb_t[:, m:m + 1])
            # depthwise causal conv(4) chunk (silu later): tensor_scalar
            # tap-multiplies (4x mode) + tensor_tensor adds (2x).
            base = nch * 512  # xa_pad column base (pad offset 3 built in)
            for i in range(4):
                t0 = cvp.tile([128, 512], BF16, name="t0", tag="convt0")
                nc.vector.tensor_scalar_mul(out=t0[:], in0=xa_pad[i][:, base:base + 512],
                                            scalar1=convw_t[i][:, 0:1])
                t1 = cvp.tile([128, 512], BF16, name="t1", tag="convt1")
                nc.vector.tensor_scalar_mul(out=t1[:], in0=xa_pad[i][:, base + 1:base + 513],
                                            scalar1=convw_t[i][:, 1:2])
                nc.vector.tensor_tensor(out=t0[:], in0=t0[:], in1=t1[:], op=ALU.add)
                nc.vector.tensor_scalar_mul(out=t1[:], in0=xa_pad[i][:, base + 2:base + 514],
                                            scalar1=convw_t[i][:, 2:3])
                t2 = cvp.tile([128, 512], BF16, name="t2", tag="convt2")
                nc.vector.tensor_scalar_mul(out=t2[:], in0=xa_pad[i][:, base + 3:base + 515],
                                            scalar1=convw_t[i][:, 3:4])
                nc.vector.tensor_tensor(out=t1[:], in0=t1[:], in1=t2[:], op=ALU.add)
                nc.vector.tensor_tensor(out=u_t[i][:, nch * 512:(nch + 1) * 512],
                                        in0=t0[:], in1=t1[:], op=ALU.add)

        # u = silu(u + convb): on the critical path into xproj, batched so the
        # scheduler pays at most one table round-trip.
        for i in range(4):
            nc.scalar.activation(out=u_t[i][:], in_=u_t[i][:], func=AF.Silu,
                                 bias=convb_t[i][:])

        # ---- xproj -> dt_lr (partitions 0:16), B (16:32), C (32:48)
        dbc_all = big.tile([48, L], BF16, name="dbc_all")
        for nch in range(4):
            sl = slice(nch * 512, (nch + 1) * 512)
            dbc_p = ps.tile([48, 512], F32, name="dbc_p", tag="mm")
            for i in range(4):
                nc.tensor.matmul(dbc_p[:], lhsT=xproj_t[i][:], rhs=u_t[i][:, sl],
                                 start=(i == 0), stop=(i == 3))
            nc.scalar.activation(out=dbc_all[:, sl], in_=dbc_p[:], func=AF.Copy)
            # bounce B/C rows through DRAM so they can be partition-broadcast
            # back by pure DMA (stride-0 partition APs) during the scan.
            nc.sync.dma_start(out=BmD[:, sl], in_=dbc_all[16:32, sl])
            nc.sync.dma_start(out=CmD[:, sl], in_=dbc_all[32:48, sl])

        # ---- dt = softplus(dt_w @ dt_lr + dt_b)  [512, L] bf16
        dt_t = [big.tile([128, L], BF16, name=f"dt{i}") for i in range(4)]
        dtu_t = [big.tile([128, L], BF16, name=f"dtu{i}") for i in range(4)]
        for m in range(4):
            for nch in range(4):
                sl = slice(nch * 512, (nch + 1) * 512)
                p = ps.tile([128, 512], F32, name="dt_p", tag="mm")
                nc.tensor.matmul(p[:], lhsT=dtw_t[:, m * 128:(m + 1) * 128],
                                 rhs=dbc_all[0:16, sl], start=True, stop=True)
                # softplus(v) = ln(1 + exp(v)) (no softplus in the ACT tables;
                # v stays small here so exp cannot overflow)
                e_s = scr.tile([128, 512], F32, name="e_s", tag="e_s")
                nc.scalar.activation(out=e_s[:], in_=p[:], func=AF.Exp,
                                     bias=dtb_t[m][:])
                nc.scalar.activation(out=dt_t[m][:, sl], in_=e_s[:], func=AF.Ln,
                                     bias=1.0)
        for m in range(4):
            nc.vector.tensor_tensor(out=dtu_t[m][:], in0=dt_t[m][:], in1=u_t[m][:],
                                    op=ALU.mult)

        # ---- selective scan, full-L n-outer: per state n, DMA-broadcast the
        # B/C rows once, then per d-block do exp/mult/scan/mult/accumulate as
        # single [128, L] ops.  Scan scratch reuses the (dead) xa_pad slots.
        y_acc = [big.tile([128, L], BF16, name=f"ya{i}") for i in range(4)]
        for n in range(16):
            bb = bc.tile([128, L], BF16, name="bbF", tag=f"bbF{n % 2}")
            nc.sync.dma_start(out=bb[:], in_=bass.AP(BmD, n * L, [[0, 128], [1, L]]))
            cb = bc.tile([128, L], BF16, name="cbF", tag=f"cbF{n % 2}")
            nc.sync.dma_start(out=cb[:], in_=bass.AP(CmD, n * L, [[0, 128], [1, L]]))
            for dblk in range(4):
                par = (n * 4 + dblk) % 2
                dA = big.tile([128, L], BF16, name="dA_s", tag=f"da{(n * 4 + dblk) % 3}")
                nc.scalar.activation(out=dA[:], in_=dt_t[dblk][:], func=AF.Exp,
                                     scale=A_t[dblk][:, n:n + 1])
                dBu = big.tile([128, L], BF16, name="dBu_s", tag=f"xap{par}")
                nc.vector.tensor_tensor(out=dBu[:], in0=dtu_t[dblk][:], in1=bb[:],
                                        op=ALU.mult)
                h = big.tile([128, L], BF16, name="h_s", tag=f"xap{2 + par}")
                nc.vector.tensor_tensor_scan(
                    out=h[:], data0=dA[:], data1=dBu[:],
                    initial=0.0, op0=ALU.mult, op1=ALU.add)
                if n == 0:
                    nc.vector.tensor_tensor(out=y_acc[dblk][:], in0=h[:],
                                            in1=cb[:], op=ALU.mult)
                else:
                    hc = big.tile([128, L], BF16, name="hc_s", tag=f"hcs{par}")
                    nc.vector.tensor_tensor(out=hc[:], in0=h[:], in1=cb[:],
                                            op=ALU.mult)
                    nc.vector.tensor_tensor(out=y_acc[dblk][:], in0=y_acc[dblk][:],
                                            in1=hc[:], op=ALU.add)

        # sz = silu(z), forced late via a zero "trigger" bias that depends on
        # y_acc (keeps the ACT table set stable during the scan's exp storm).
        trig = wpool.tile([128, 1], F32, name="trig")
        nc.vector.tensor_scalar_mul(out=trig[:], in0=y_acc[0][:, 0:1], scalar1=0.0)
        for m in range(4):
            nc.scalar.activation(out=sz[m][:], in_=sz[m][:], func=AF.Silu,
                                 bias=trig[:])

        # ---- y = (y_acc + Dp*u) * silu(z) (in place); out = out_w @ y + x
        yg = y_acc
        for m in range(4):
            nc.vector.scalar_tensor_tensor(
                out=y_acc[m][:], in0=u_t[m][:], scalar=Dp_t[m][:], in1=y_acc[m][:],
                op0=ALU.mult, op1=ALU.add)
            nc.vector.tensor_tensor(out=y_acc[m][:], in0=y_acc[m][:], in1=sz[m][:],
                                    op=ALU.mult)
        for m in range(2):
            for nch in range(4):
                sl = slice(nch * 512, (nch + 1) * 512)
                p = ps.tile([128, 512], F32, name="out_p", tag="mm")
                for i in range(4):
                    nc.tensor.matmul(p[:], lhsT=outw_t[i][:, m * 128:(m + 1) * 128],
                                     rhs=yg[i][:, sl], start=(i == 0), stop=(i == 3))
                o = sc.tile([128, 512], F32, name="o", tag="outsc")
                nc.vector.scalar_tensor_tensor(
                    out=o[:], in0=p[:], scalar=1.0, in1=x_t[m][:, sl],
                    op0=ALU.mult, op1=ALU.add)
                nc.sync.dma_start(out=outT[m, :, sl], in_=o[:])
        stack.close()
    with _restrict_act_tables({"natural_log_exp_and_others", "silu_and_others"}):
        nc.compile()
    return nc


def prep_phase1_inputs(inputs, xs, core):
    b, k = divmod(core, 4)
    wkey = ("p1w", k, id(inputs['in_w']), id(inputs['A_log']))
    if wkey in _cache:
        d = dict(_cache[wkey])
        d["xT"] = np.ascontiguousarray(xs[b, k]).reshape(2, 128, L).astype(np.float32)
        return d
    A = -np.exp(inputs['A_log'][k]).astype(np.float32)          # [512, 16]
    inb_full = (inputs['in_w'][k].astype(np.float32)
                @ inputs['norm_b'][k].astype(np.float32))       # [1024]
    w = {
        "nwb": np.stack([inputs['norm_w'][k].reshape(2, 128),
                         inputs['norm_b'][k].reshape(2, 128)], 2).astype(np.float32),
        "inwT": _bf16(inputs['in_w'][k].T.reshape(2, 128, 2 * D_INNER)),
        "convw": inputs['conv_w'][k][:, 0, :].reshape(4, 128, D_CONV).astype(np.float32),
        "convb": inputs['conv_b'][k].reshape(4, 128, 1).astype(np.float32),
        "xprojT": _bf16(inputs['xproj_w'][k].T.reshape(4, 128, 48)),
        "dtwT": _bf16(inputs['dt_w'][k].T),
        "dtb": inputs['dt_b'][k].reshape(4, 128, 1).astype(np.float32),
        "AT": A.reshape(4, 128, D_STATE),
        "Dpt": inputs['Dp'][k].reshape(4, 128, 1).astype(np.float32),
        "outwT": _bf16(inputs['out_w'][k].T.reshape(4, 128, C)),
        "inb": inb_full.reshape(8, 128).T.astype(np.float32).copy(),
        "oneM": _bf16(np.full((128, 128), 1.0 / 256.0)),
    }
    _cache[wkey] = w
    d = dict(w)
    d["xT"] = np.ascontiguousarray(xs[b, k]).reshape(2, 128, L).astype(np.float32)
    return d


def run_phase1_bass(inputs, xs):
    if "p1" not in _cache:
        _cache["p1"] = build_phase1()
    nc = _cache["p1"]
    ins = [prep_phase1_inputs(inputs, xs, core) for core in range(8)]
    res = run_bass_kernel_spmd(nc, ins, list(range(8)))
    return [res.results[c]["outT"].reshape(C, L) for c in range(8)]


# ---------------------------------------------------------------------------
# numpy reference fallback (kept for testing)
# ---------------------------------------------------------------------------

def _sigmoid(v):
    return 1.0 / (1.0 + np.exp(-v))


def mamba_block_np(xT, nw, nb, in_w, conv_w, conv_b, xproj_w, dt_w, dt_b,
                   A_log, Dp, out_w):
    x = xT.T
    mu = x.mean(-1, keepdims=True)
    var = ((x - mu) ** 2).mean(-1, keepdims=True)
    h = (x - mu) / np.sqrt(var + EPS) * nw + nb
    xz = h @ in_w.T
    xa, z = xz[:, :D_INNER], xz[:, D_INNER:]
    xa_t = xa.T
    w = conv_w[:, 0, :]
    pad = np.pad(xa_t, ((0, 0), (D_CONV - 1, 0)))
    conv = sum(pad[:, i:i + L] * w[:, i:i + 1] for i in range(D_CONV))
    u_t = conv + conv_b[:, None]
    u_t = u_t * _sigmoid(u_t)
    u = u_t.T
    dbc = u @ xproj_w.T
    dt_lr = dbc[:, :DT_RANK]
    Bm = dbc[:, DT_RANK:DT_RANK + D_STATE]
    Cm = dbc[:, DT_RANK + D_STATE:]
    vv = dt_lr @ dt_w.T + dt_b
    dt = np.log1p(np.exp(-np.abs(vv))) + np.maximum(vv, 0)
    A = -np.exp(A_log)
    dA = np.exp(dt[:, :, None] * A[None])
    dBu = dt[:, :, None] * Bm[:, None, :] * u[:, :, None]
    hs = np.zeros((D_INNER, D_STATE), np.float32)
    ys = np.zeros((L, D_INNER), np.float32)
    for t in range(L):
        hs = dA[t] * hs + dBu[t]
        ys[t] = (hs * Cm[t][None, :]).sum(-1)
    y = ys + Dp * u
    y = y * (z * _sigmoid(z))
    mo = y @ out_w.T
    return xT + mo.T


def run_phase1_np(inputs, xs):
    outs = []
    for core in range(8):
        b, k = divmod(core, 4)
        outs.append(mamba_block_np(
            np.ascontiguousarray(xs[b, k]), inputs['norm_w'][k], inputs['norm_b'][k],
            inputs['in_w'][k], inputs['conv_w'][k], inputs['conv_b'][k],
            inputs['xproj_w'][k], inputs['dt_w'][k], inputs['dt_b'][k],
            inputs['A_log'][k], inputs['Dp'][k], inputs['out_w'][k]))
    return outs


# ---------------------------------------------------------------------------
# top level
# ---------------------------------------------------------------------------

def kernel(**inputs):
    inputs = {k: np.asarray(v, np.float32) if np.asarray(v).dtype == np.float32
              else np.asarray(v) for k, v in inputs.items()}
    xs = scan_jego_np(inputs['feat0'], inputs['feat1'])  # [B,4,C,L]
    p1 = run_phase1(inputs, xs)
    ys = np.stack([np.stack(p1[4 * b:4 * b + 4], 0) for b in range(B)], 0)
    d0, d1 = merge_jego_np(ys, H8, W8)
    Dfull = np.concatenate([d0, d1], 0)
    desc = run_phase2(Dfull, inputs['glu_w'], inputs['glu_b'])
    dd0, dd1 = desc[:B], desc[B:]
    return np.stack([dd0.reshape(B, C, -1), dd1.reshape(B, C, -1)], 0).astype(np.float32)


def run_phase1(inputs, xs):
    return run_phase1_bass(inputs, xs)

